# revision 23
# baseline (speedup 1.0000x reference)
"""Trainium2 Bass kernel for nn_CompNet (spiking LIF RNN).

Math summary (reformulation of the reference):
  Per step t:  h = W1 x_t + b1;  i = Wr [h; y] + br
               v1 <- 0.5 v1 + 0.5 i ; s1 = (v1>=1); v1 *= (1-s1)
               logits = W2 s1 + b2
               v2 <- 0.5 v2 + 0.5 logits ; s2 = (v2>=1); v2 *= (1-s2)
  out = mean_{t>=15} s2                                    -> (B, C)

Key algebraic folds (all host-side, exact in fp32):
  * h only enters via Wr_h @ h, so fold:  Wtil = 0.5*Wr_h@W1   (64x700)
  * substitute s = 1 - m with m = (v < 1), folding the constant
    Wr_y@1 / W2@1 terms into per-population biases:
       bt1 = 0.5*(Wr_h b1 + br + Wr_y 1),  bt2 = 0.5*(b2 + W2 1)
  * LIF1 (rows 0..63) and LIF2 (rows 64..83) are stacked into one 84-row
    population, with LIF2 lagging one step (its drive only needs s1 of the
    previous loop iteration).

Per-core execution (feature-major, batch on the free axis, B_local=32):
  bigmm (PE):  psA block b [84,512] = Wt@x for 16 steps (6 matmuls; the
               bias rides two spare contraction rows as a double-bf16
               split against constant-1 rows of x, so psum = drive+bias)
  loop j (PE): psA slice [84,32] += L@Mbuf[0:64, blk j]  (1 matmul, acc)
  loop j (DVE): m*_j = (0.5*cu_{j-1} < psum_j) -> Mbuf blk j+1
                v_j  = 0.5*cu_{j-1} - psum_j
                cu_j = (v_j + 1)*m*_j
                S   += Mbuf[64:84, blk j-2]   (one hidden stat add/iter)
  Output: out = (S - 117.5)*(-2/235)

Sync strategy: walrus accepts ONE wait per compute instruction.  Each
instruction keeps exactly the one wait that is not transitively covered
by engine-order (PE/ACT/DVE streams are in-order); see _fix_sync.

Sharding: pure data parallelism, batch 256 -> 8 cores x 32.
"""

import numpy as np
import ml_dtypes

BF16 = ml_dtypes.bfloat16

B, T, D, H, C = 256, 250, 700, 64, 20
NCORES = 8
BL = B // NCORES          # 32 batch per core
P = H + C                 # 84 stacked feature rows
KCH = 6                   # ceil(700/128) contraction chunks
DP = KCH * 128            # 768 padded feature dim
NCOL = T * BL             # 8000 drive columns per core
BIAS_ROW = 704            # 64-aligned bias rows inside the padded contraction
VTH_INIT = 2.0e9          # suppresses the phantom LIF2 step at j=0
CH_COLS = [512, 1024, 2048, 2048, 2048, 320]   # x DMA chunks (sum 8000)
NBLK = NCOL // 512 + 1    # 16 psA blocks (last is 320 cols)
RCH = 47                  # reduction chunk (5 x 47 = 235 stat blocks)

_CACHE = {}
SIM_SAFE_STOPS = False    # True: stop every psum slice (CoreSim read lint)


def _build_nc():
    import concourse.bass as bass
    import concourse.mybir as mybir
    from concourse.tile import TileContext

    dt = mybir.dt
    AF = mybir.ActivationFunctionType
    OP = mybir.AluOpType
    ts = bass.ts

    # detect_race_conditions=False: the hand-managed single-wait sync (see
    # _fix_sync) relies on engine-order transitivity the simulator's race
    # detector cannot see.
    nc = bass.Bass(
        "TRN2", target_bir_lowering=False, debug=False,
        detect_race_conditions=False,
    )

    xT = nc.dram_tensor("xT", [KCH, 128, NCOL], dt.bfloat16, kind="ExternalInput").ap()
    Wt = nc.dram_tensor("Wt", [KCH, 128, P], dt.bfloat16, kind="ExternalInput").ap()
    Lw = nc.dram_tensor("Lw", [H, P], dt.bfloat16, kind="ExternalInput").ap()
    out_d = nc.dram_tensor("out", [C, BL], dt.float32, kind="ExternalOutput").ap()

    # chunk start columns
    ch_start = [0]
    for w in CH_COLS:
        ch_start.append(ch_start[-1] + w)

    loop_stt_names = []
    v_names = []
    stat_add_names = []
    tail_mm_names = []
    loop_mm_names = []
    big_mm_names = []

    with TileContext(nc) as tc:
        with (
            tc.tile_pool(name="const", bufs=1) as cp,
            tc.tile_pool(name="xs", bufs=3) as xp,
            tc.tile_pool(name="wk", bufs=4) as wp,
            tc.tile_pool(name="psA", bufs=6, space="PSUM") as psA,
            tc.tile_pool(name="psB", bufs=1, space="PSUM") as psB,
        ):
            # ---- persistent tiles ----
            M_t = cp.tile([P, NCOL + 2 * BL], dt.bfloat16, tag="M")    # blocks 0..251
            wts = [
                cp.tile([128, P], dt.bfloat16, tag=f"w{k}", name=f"wts{k}")
                for k in range(KCH)
            ]
            L_t = cp.tile([H, P], dt.bfloat16, tag="L")
            cu0 = cp.tile([P, BL], dt.float32, tag="cu0")
            ones2 = cp.tile([128, BL], dt.bfloat16, tag="ones2")
            S_t = cp.tile([P, BL], dt.float32, tag="S")
            R_t = cp.tile([P, BL], dt.float32, tag="R")

            # ---- prologue: weights, inits ----
            for k in range(KCH):
                nc.sync.dma_start(out=wts[k][:, :], in_=Wt[k, :, :])
            nc.sync.dma_start(out=L_t[:, :], in_=Lw[:, :])

            nc.vector.memset(M_t[0:H, 0:BL], 1.0)     # m_{-1} = 1 (y=0)
            nc.vector.memset(M_t[H:P, 0:BL], 0.0)
            nc.vector.memset(cu0[0:H, :], 0.0)        # v1 carry starts at 0
            nc.vector.memset(S_t[H:P, :], 0.0)        # output statistic
            nc.vector.memset(cu0[H:P, :], VTH_INIT)   # kill phantom LIF2 step
            nc.vector.memset(ones2[64:66, :], 1.0)    # bias rows' rhs (j=250)

            # ---- x DMAs ----
            xtiles = {}

            def emit_xdma(c):
                c0, w = ch_start[c], CH_COLS[c]
                for k in range(KCH):
                    t = xp.tile([128, 2048], dt.bfloat16, tag=f"x{k}")
                    nc.sync.dma_start(out=t[:, 0:w], in_=xT[k, :, c0:c0 + w])
                    xtiles[(c, k)] = t

            # ---- big matmul, 512-col psA blocks ----
            pa_tiles = {}

            def bigmm_block_ops(b):
                """Yield 12 matmul thunks (6 K-chunks x 2 col-halves) for
                psA block b.  Halves keep each PE instruction ~350ns so the
                interleave never stalls the loop's matmul for long.  Only
                the very first carries start=True: the bank-wide
                pending-zero makes the other half's first write zeroing."""
                col0 = 512 * b
                nw = min(512, NCOL - col0)
                c = next(i for i in range(len(CH_COLS))
                         if ch_start[i] <= col0 < ch_start[i + 1])
                n0 = col0 - ch_start[c]
                pa = psA.tile([P, 512], dt.float32, tag="pa")
                pa_tiles[b] = (pa, nw)
                h1 = min(256, nw)

                def mk_mm(k, h, pa=pa, n0=n0, nw=nw, c=c, h1=h1):
                    o0 = 0 if h == 0 else h1
                    ow = h1 if h == 0 else nw - h1

                    def f():
                        if ow <= 0:
                            return
                        i = nc.tensor.matmul(
                            out=pa[:, o0:o0 + ow], lhsT=wts[k][:, :],
                            rhs=xtiles[(c, k)][:, n0 + o0:n0 + o0 + ow],
                            start=(k == 0 and h == 0), stop=False,
                            skip_group_check=not (k == 0 and h == 0),
                        )
                        big_mm_names.append(i.ins.name)
                    return f

                for k in range(KCH):
                    for h in (0, 1):
                        yield mk_mm(k, h)

            # ---- interleave schedule ----
            extras = {}
            for b in range(1, NBLK):
                for i, th in enumerate(bigmm_block_ops(b)):
                    extras.setdefault(16 * b - 14 + i, []).append(th)
            extras.setdefault(20, []).append(lambda: emit_xdma(3))
            extras.setdefault(48, []).append(lambda: emit_xdma(4))
            extras.setdefault(112, []).append(lambda: emit_xdma(5))

            # prologue: x chunks 0..2 + psA block 0
            emit_xdma(0)
            emit_xdma(1)
            emit_xdma(2)
            for th in bigmm_block_ops(0):
                th()

            # ---- the sequential LIF loop ----
            cu_prev = cu0
            for j in range(T + 1):
                for th in extras.pop(j, []):
                    th()
                if j < T:
                    pa, _ = pa_tiles[j // 16]
                    ps = pa[:, (j % 16) * BL:(j % 16) * BL + BL]
                else:
                    # last iteration: bias-only drive via the two bias rows
                    pb = psB.tile([P, BL], dt.float32, tag="pb")
                    ib = nc.tensor.matmul(
                        out=pb[:, :], lhsT=wts[KCH - 1][64:66, :],
                        rhs=ones2[64:66, :], start=True, stop=False,
                    )
                    tail_mm_names.append(ib.ins.name)
                    ps = pb[:, :]
                # stop only on the bank's true last slice: stop flushes the
                # whole 512-col bank (~+120ns per matmul if set every slice).
                # CoreSim's read lint wants stop before any psum read; the
                # sim build sets it every slice (no data effect either way:
                # psum zeroing is per-byte pending-zero, set only by start).
                last_slice = j >= T or j % 16 == 15 or j == T - 1
                im = nc.tensor.matmul(
                    out=ps, lhsT=L_t[:, :], rhs=M_t[0:H, ts(j, BL)],
                    start=False, stop=SIM_SAFE_STOPS or last_slice,
                    skip_group_check=(j < T and j % 16 != 0),
                )
                loop_mm_names.append(im.ins.name)
                # qsum = 1 - (drive + recurrent); spike mask straight off
                # PSUM in ONE fused op:  v < 1  <=>  0.5*cu < qsum.
                i1 = nc.vector.scalar_tensor_tensor(
                    out=M_t[:, ts(j + 1, BL)], in0=cu_prev[:, :], scalar=0.5,
                    in1=ps, op0=OP.mult, op1=OP.is_lt,
                )
                loop_stt_names.append(i1.ins.name)
                if j < T:
                    v = wp.tile([P, BL], dt.float32, tag="v")
                    # u = v - 1 = 0.5*cu - qsum
                    i2 = nc.vector.scalar_tensor_tensor(
                        out=v[:, :], in0=cu_prev[:, :], scalar=0.5,
                        in1=ps, op0=OP.mult, op1=OP.subtract,
                    )
                    cu = wp.tile([P, BL], dt.float32, tag="cu")
                    # cu = v*m = (u + 1)*m
                    i3 = nc.vector.scalar_tensor_tensor(
                        out=cu[:, :], in0=v[:, :], scalar=1.0,
                        in1=M_t[:, ts(j + 1, BL)], op0=OP.add, op1=OP.mult,
                    )
                    loop_stt_names.extend([i2.ins.name, i3.ins.name])
                    v_names.append(i2.ins.name)
                    cu_prev = cu
                if 19 <= j <= 250:
                    bk = j - 2      # stat blocks 17..248 inside the loop
                    i4 = nc.vector.tensor_tensor(
                        out=S_t[H:P, :], in0=S_t[H:P, :],
                        in1=M_t[H:P, ts(bk, BL)], op=OP.add,
                    )
                    stat_add_names.append(i4.ins.name)
            for jj in sorted(extras):
                for th in extras[jj]:
                    th()

            # ---- tail: last 3 stat blocks, then the output scale ----
            for bk in (249, 250, 251):
                nc.vector.tensor_tensor(
                    out=S_t[H:P, :], in0=S_t[H:P, :],
                    in1=M_t[H:P, ts(bk, BL)], op=OP.add,
                )
            nc.vector.tensor_scalar(
                out=R_t[H:P, :], in0=S_t[H:P, :],
                scalar1=235.0, scalar2=-1.0 / 235.0,
                op0=OP.subtract, op1=OP.mult,
            )
            nc.sync.dma_start(out=out_d[:, :], in_=R_t[H:P, :])

    nc._loop_stt_names = loop_stt_names + stat_add_names
    nc._v_names = v_names
    nc._tail_mm_names = tail_mm_names
    nc._loop_mm_names = loop_mm_names
    nc._big_mm_names = big_mm_names
    _fix_sync(nc)
    return nc


def _eng_of(w):
    n = w.ant_name
    if "DVE" in n:
        return "DVE"
    if "Activation" in n:
        return "ACT"
    if "PE" in n:
        return "PE"
    if "Pool" in n:
        return "POOL"
    return "OTHER"


def _fix_sync(nc):
    """walrus accepts only ONE sync wait per compute instruction (AC/MM/STT).
    Keep, per instruction, the single wait that engine-order transitivity
    cannot cover:

      * loop STTs (mask/v/cu) and stat adds: the mask keeps its PE (psum)
        wait; the others drop their same-engine self-waits (DVE executes in
        order, and consecutive [84,32] ops observe each other's writes at
        issue cadence -- the baseline already relied on this for mask<-cu).
      * loop matmuls: keep the DVE wait (recurrent-mask RAW).  The psum
        accumulation group ordering vs the bigmm matmuls is PE-in-order.
      * bigmm matmuls: keep the DMA (xtile) wait.  The psA-slot WAR vs the
        DVE readers of 4 blocks ago is covered by the kept DVE waits of the
        loop matmuls that precede this matmul in the PE stream (DVE
        semaphore counts are completion-ordered), with ~50 periods margin.
      * tail bias matmul: keep the DVE (ones2 memset) wait; its weight DMA
        is covered by the prologue LDWEIGHTS of the same tile.
      * Drains keep only output-DMA lanes (input-DMA completions are covered
        by their consumers' waits; engine completion by the final barrier).
    """
    import concourse.mybir as mybir

    tail_mm = set(nc._tail_mm_names)
    loop_mm = set(nc._loop_mm_names)
    big_mm = set(nc._big_mm_names)
    loop_stt = set(nc._loop_stt_names)

    out_names = set()
    for alloc in nc.m.functions[0].allocations:
        if (
            isinstance(alloc, mybir.MemoryLocationSet)
            and alloc.kind == "ExternalOutput"
        ):
            for ml in alloc.memorylocations:
                out_names.add(ml.name)
    keep_lanes = set()
    for name, inst in nc.inst_map.items():
        if "DMA" not in type(inst).__name__:
            continue
        c = inst.concise()
        if any(f"@{n}" in c.split("in=")[0] for n in out_names):
            for u in (inst.sync_info.on_update or []) if inst.sync_info else []:
                keep_lanes.add(u.ant_name)

    problems = []
    for name, inst in nc.inst_map.items():
        si = inst.sync_info
        if si is None or not si.on_wait:
            continue
        waits = list(si.on_wait)
        own = {u.ant_name for u in (si.on_update or [])}

        if name in loop_mm or name in tail_mm:
            kept = [w for w in waits if _eng_of(w) == "DVE"]
            if not kept:
                kept = [w for w in waits if w.ant_name not in own]
        elif name in big_mm:
            kept = [w for w in waits
                    if _eng_of(w) not in ("DVE", "ACT")
                    and w.ant_name not in own]
            if not kept and waits:
                kept = [w for w in waits if w.ant_name not in own][:1]
        elif name in loop_stt:
            kept = [w for w in waits if w.ant_name not in own]
        elif len(waits) >= 2:
            kept = [w for w in waits if w.ant_name not in own]
            if "Drain" in type(inst).__name__ and len(kept) > 1:
                kept = [w for w in kept if w.ant_name in keep_lanes]
        else:
            continue

        if len(kept) != len(waits):
            si.on_wait = kept
        if len(kept) > 1 and "Drain" not in type(inst).__name__ \
                and "DMA" not in type(inst).__name__ \
                and "Branch" not in type(inst).__name__:
            problems.append((name, type(inst).__name__,
                             [w.ant_name for w in kept]))
    if problems:
        for p in problems[:8]:
            print("MULTIWAIT:", p)

def _prep_shared(W1, b1, Wr, br, W2, b2):
    f32 = np.float32
    W1 = np.asarray(W1, f32); b1 = np.asarray(b1, f32)
    Wr = np.asarray(Wr, f32); br = np.asarray(br, f32)
    W2 = np.asarray(W2, f32); b2 = np.asarray(b2, f32)
    Wrh, Wry = Wr[:, :H], Wr[:, H:]
    # Negated ("qsum = 1 - v") encoding: psum = (1-bt) - Wtil@x
    # - 0.5*[Wry;W2]@m with m in {0,1}; spike test is then 0.5*cu < q.
    Wtil = -0.5 * (Wrh @ W1)                                  # [64, 700]
    bt1 = 0.5 * (Wrh @ b1 + br + Wry.sum(axis=1))
    bt2 = 0.5 * (b2 + W2.sum(axis=1))
    bfl = 1.0 - np.concatenate([bt1, bt2])                    # [84] fp32
    Wtp = np.zeros((P, DP), f32)
    Wtp[:H, :D] = Wtil
    # Bias rides two spare contraction rows as a double-bf16 split; the
    # matching x rows are constant 1.0, so psum picks up ~fp32 bias.
    bhi = bfl.astype(BF16).astype(f32)
    Wtp[:, BIAS_ROW] = bhi
    Wtp[:, BIAS_ROW + 1] = bfl - bhi
    Wt6 = np.ascontiguousarray(
        Wtp.reshape(P, KCH, 128).transpose(1, 2, 0)
    ).astype(BF16)                                            # [6, 128, 84]
    L = np.concatenate([0.5 * Wry.T, 0.5 * W2.T], axis=1).astype(BF16)
    return Wt6, L


def _ensure_ntff_hook():
    """The RL container's antenv stub lacks axon_hooks; bass_utils imports it
    unconditionally when tracing. Register the ctypes-based hook ourselves."""
    import sys
    import types
    try:
        import antenv
        if "antenv.axon_hooks" in sys.modules:
            return
        mod = types.ModuleType("antenv.axon_hooks")
        _h = [None]
        mod.set_axon_ntff_profile_hook = lambda h: _h.__setitem__(0, h)
        mod.get_axon_ntff_profile_hook = lambda: _h[0]
        sys.modules["antenv.axon_hooks"] = mod
        antenv.axon_hooks = mod
        try:
            from trn_agent_boot.trn_boot import _ntff_profile_via_ctypes
            mod.set_axon_ntff_profile_hook(
                _ntff_profile_via_ctypes("/opt/axon/libaxon_pjrt.so")
            )
        except Exception:
            pass
    except Exception:
        pass


def kernel(x, W1, b1, Wr, br, W2, b2):
    from concourse.bass_utils import run_bass_kernel_spmd

    _ensure_ntff_hook()

    if "nc" not in _CACHE:
        _CACHE["nc"] = _build_nc()
    nc = _CACHE["nc"]

    Wt6, L = _prep_shared(W1, b1, Wr, br, W2, b2)

    x = np.asarray(x, np.float32)
    xbf = x.astype(BF16)                                      # (B, T, D)
    in_maps = []
    for c in range(NCORES):
        xc = xbf[c * BL:(c + 1) * BL]                         # (32, 250, 700)
        xt = np.zeros((DP, T, BL), BF16)
        xt[:D] = xc.transpose(2, 1, 0)                        # (d, t, b)
        xt[BIAS_ROW:BIAS_ROW + 2] = 1.0                       # bias rows
        in_maps.append({
            "xT": np.ascontiguousarray(xt.reshape(KCH, 128, NCOL)),
            "Wt": Wt6, "Lw": L,
        })

    res = run_bass_kernel_spmd(nc, in_maps, core_ids=list(range(NCORES)))
    _CACHE["last_results"] = res
    out = np.concatenate(
        [np.asarray(r["out"]).T for r in res.results], axis=0
    ).astype(np.float32)                                      # (256, 20)
    return out


# revision 25
# speedup vs baseline: 1.2221x; 1.2221x over previous
"""Trainium2 Bass kernel for nn_CompNet (spiking LIF RNN).

Math summary (reformulation of the reference):
  Per step t:  h = W1 x_t + b1;  i = Wr [h; y] + br
               v1 <- 0.5 v1 + 0.5 i ; s1 = (v1>=1); v1 *= (1-s1)
               logits = W2 s1 + b2
               v2 <- 0.5 v2 + 0.5 logits ; s2 = (v2>=1); v2 *= (1-s2)
  out = mean_{t>=15} s2                                    -> (B, C)

Key algebraic folds (all host-side, exact in fp32):
  * h only enters via Wr_h @ h, so fold:  Wtil = 0.5*Wr_h@W1   (64x700)
  * substitute s = 1 - m with m = (v < 1), folding the constant
    Wr_y@1 / W2@1 terms into per-population biases:
       bt1 = 0.5*(Wr_h b1 + br + Wr_y 1),  bt2 = 0.5*(b2 + W2 1)
  * LIF1 (rows 0..63) and LIF2 (rows 64..83) are stacked into one 84-row
    population, with LIF2 lagging one step (its drive only needs s1 of the
    previous loop iteration).

Per-core execution (feature-major, batch on the free axis, B_local=32):
  bigmm (PE):  psA block b [84,512] = Wt@x for 16 steps (6 matmuls; the
               bias rides two spare contraction rows as a double-bf16
               split against constant-1 rows of x, so psum = drive+bias)
  loop j (PE): psA slice [84,32] += L@Mbuf[0:64, blk j]  (1 matmul, acc)
  loop j (DVE): m*_j = (0.5*cu_{j-1} < psum_j) -> Mbuf blk j+1
                v_j  = 0.5*cu_{j-1} - psum_j
                cu_j = (v_j + 1)*m*_j
                S   += Mbuf[64:84, blk j-2]   (one hidden stat add/iter)
  Output: out = (S - 117.5)*(-2/235)

Sync strategy: walrus accepts ONE wait per compute instruction.  Each
instruction keeps exactly the one wait that is not transitively covered
by engine-order (PE/ACT/DVE streams are in-order); see _fix_sync.

Sharding: pure data parallelism, batch 256 -> 8 cores x 32.
"""

import numpy as np
import ml_dtypes

BF16 = ml_dtypes.bfloat16

B, T, D, H, C = 256, 250, 700, 64, 20
NCORES = 8
BL = B // NCORES          # 32 batch per core
P = H + C                 # 84 stacked feature rows
KCH = 6                   # ceil(700/128) contraction chunks
DP = KCH * 128            # 768 padded feature dim
NCOL = T * BL             # 8000 drive columns per core
BIAS_ROW = 704            # 64-aligned bias rows inside the padded contraction
VTH_INIT = 2.0e9          # suppresses the phantom LIF2 step at j=0
TSTEPS = T + 1            # 251 loop steps incl the bias-only last one
GEN = 64                  # steps per bank generation (4 banks x 16 cols)
NBGEN = 4                 # psum banks per generation (groups alternate)

# Steps are permuted so consecutive iterations hit different psum banks:
# step j = 64g + 4c + r lives in block q=4g+r (bank 4*(g%2)+r), column c.
# That keeps each matmul's bank-WAR 4 iterations stale, so its only live
# dependency is the previous mask (the recurrent-spike RAW).
def _slice_of(j):
    g, o = divmod(j, GEN)
    return NBGEN * g + o % NBGEN, o // NBGEN

NGENS = (TSTEPS + GEN - 1) // GEN                      # 4
NBLK = NBGEN * NGENS                                   # 16 blocks
BLK_W = [0] * NBLK                                     # cols (steps) per block
for _j in range(TSTEPS):
    BLK_W[_slice_of(_j)[0]] += 1
BLK_S = [0] * (NBLK + 1)                               # start col-group
for _q in range(NBLK):
    BLK_S[_q + 1] = BLK_S[_q] + BLK_W[_q]
NCOLP = BLK_S[NBLK] * BL                               # 8032 permuted columns
CH_BLOCKS = [[0], [1, 2, 3], [4, 5, 6, 7], [8, 9, 10, 11], [12, 13, 14, 15]]
CH_COLS = [sum(BLK_W[b] for b in grp) * BL for grp in CH_BLOCKS]

_CACHE = {}
SIM_SAFE_STOPS = False    # True: stop every psum slice (CoreSim read lint)


def _build_nc():
    import concourse.bass as bass
    import concourse.mybir as mybir
    from concourse.tile import TileContext

    dt = mybir.dt
    AF = mybir.ActivationFunctionType
    OP = mybir.AluOpType
    ts = bass.ts

    # detect_race_conditions=False: the hand-managed single-wait sync (see
    # _fix_sync) relies on engine-order transitivity the simulator's race
    # detector cannot see.
    nc = bass.Bass(
        "TRN2", target_bir_lowering=False, debug=False,
        detect_race_conditions=False,
    )

    xT = nc.dram_tensor("xT", [KCH, 128, NCOLP], dt.bfloat16, kind="ExternalInput").ap()
    Wt = nc.dram_tensor("Wt", [KCH, 128, P], dt.bfloat16, kind="ExternalInput").ap()
    Lw = nc.dram_tensor("Lw", [H, P], dt.bfloat16, kind="ExternalInput").ap()
    out_d = nc.dram_tensor("out", [C, BL], dt.float32, kind="ExternalOutput").ap()

    # chunk start columns
    ch_start = [0]
    for w in CH_COLS:
        ch_start.append(ch_start[-1] + w)

    loop_stt_names = []
    v_names = []
    stat_add_names = []
    tail_mm_names = []
    loop_mm_names = []
    big_mm_names = []

    with TileContext(nc) as tc:
        with (
            tc.tile_pool(name="const", bufs=1) as cp,
            tc.tile_pool(name="xs", bufs=3) as xp,
            tc.tile_pool(name="wk", bufs=4) as wp,
            tc.tile_pool(name="psA", bufs=8, space="PSUM") as psA,
        ):
            # ---- persistent tiles ----
            M_t = cp.tile([P, NCOL + 2 * BL], dt.bfloat16, tag="M")    # blocks 0..251
            wts = [
                cp.tile([128, P], dt.bfloat16, tag=f"w{k}", name=f"wts{k}")
                for k in range(KCH)
            ]
            L_t = cp.tile([H, P], dt.bfloat16, tag="L")
            cu0 = cp.tile([P, BL], dt.float32, tag="cu0")
            S_t = cp.tile([P, BL], dt.float32, tag="S")
            R_t = cp.tile([P, BL], dt.float32, tag="R")

            # ---- prologue: weights, inits ----
            for k in range(KCH):
                nc.sync.dma_start(out=wts[k][:, :], in_=Wt[k, :, :])
            nc.sync.dma_start(out=L_t[:, :], in_=Lw[:, :])

            nc.vector.memset(M_t[0:H, 0:BL], 1.0)     # m_{-1} = 1 (y=0)
            nc.vector.memset(M_t[H:P, 0:BL], 0.0)
            nc.vector.memset(cu0[0:H, :], 0.0)        # v1 carry starts at 0
            nc.vector.memset(S_t[H:P, :], 0.0)        # output statistic
            nc.vector.memset(cu0[H:P, :], VTH_INIT)   # kill phantom LIF2 step

            # ---- x DMAs ----
            xtiles = {}

            def emit_xdma(c):
                c0, w = ch_start[c], CH_COLS[c]
                for k in range(KCH):
                    t = xp.tile([128, 2048], dt.bfloat16, tag=f"x{k}")
                    nc.sync.dma_start(out=t[:, 0:w], in_=xT[k, :, c0:c0 + w])
                    xtiles[(c, k)] = t

            # ---- big matmul, 512-col psA blocks ----
            pa_tiles = {}

            def bigmm_block_ops(q):
                """Yield 6 matmul thunks computing psA block q (one bank,
                up to 512 cols = 16 interleaved steps)."""
                col0 = BLK_S[q] * BL
                nw = BLK_W[q] * BL
                c = next(i for i in range(len(CH_BLOCKS))
                         if q in CH_BLOCKS[i])
                n0 = col0 - sum(CH_COLS[:c])
                pa = psA.tile([P, 512], dt.float32, tag="pa")
                pa_tiles[q] = (pa, BLK_W[q])

                def mk_mm(k, pa=pa, n0=n0, nw=nw, c=c):
                    def f():
                        i = nc.tensor.matmul(
                            out=pa[:, 0:nw], lhsT=wts[k][:, :],
                            rhs=xtiles[(c, k)][:, n0:n0 + nw],
                            start=(k == 0), stop=False,
                        )
                        big_mm_names.append(i.ins.name)
                    return f

                for k in range(KCH):
                    yield mk_mm(k)

            # ---- interleave schedule ----
            extras = {}
            for g in range(1, NGENS):
                for r in range(NBGEN):
                    q = NBGEN * g + r
                    for i, th in enumerate(bigmm_block_ops(q)):
                        extras.setdefault(GEN * (g - 1) + 8 + 6 * r + i,
                                          []).append(th)
            extras.setdefault(36, []).append(lambda: emit_xdma(3))
            extras.setdefault(100, []).append(lambda: emit_xdma(4))

            # prologue: x chunks 0..2 + psA blocks 0..3 (generation 0)
            emit_xdma(0)
            emit_xdma(1)
            emit_xdma(2)
            for q in range(NBGEN):
                for th in bigmm_block_ops(q):
                    th()

            # ---- the sequential LIF loop ----
            cu_prev = cu0
            for j in range(TSTEPS):
                for th in extras.pop(j, []):
                    th()
                # stat add first in the DVE stream so the scheduler keeps
                # mask/v/cu in emission order (cu last would stall the next
                # mask through the 2-deep DVE issue pipeline).
                if 19 <= j:
                    bk = j - 2      # stat blocks 17..249 inside the loop
                    i4 = nc.vector.tensor_tensor(
                        out=S_t[H:P, :], in0=S_t[H:P, :],
                        in1=M_t[H:P, ts(bk, BL)], op=OP.add,
                    )
                    stat_add_names.append(i4.ins.name)
                q, c = _slice_of(j)
                pa, wq = pa_tiles[q]
                ps = pa[:, c * BL:(c + 1) * BL]
                # stop on the bank's last slice only (stop flushes the whole
                # bank); CoreSim's read lint wants stop before psum reads,
                # so the sim build stops every slice (no data effect).
                im = nc.tensor.matmul(
                    out=ps, lhsT=L_t[:, :], rhs=M_t[0:H, ts(j, BL)],
                    start=False, stop=SIM_SAFE_STOPS or c == wq - 1,
                    skip_group_check=(c != 0),
                )
                loop_mm_names.append(im.ins.name)
                # qsum = 1 - (drive + recurrent); spike mask straight off
                # PSUM in ONE fused op:  v < 1  <=>  0.5*cu < qsum.
                i1_ = nc.vector.scalar_tensor_tensor(
                    out=M_t[:, ts(j + 1, BL)], in0=cu_prev[:, :], scalar=0.5,
                    in1=ps, op0=OP.mult, op1=OP.is_lt,
                )
                loop_stt_names.append(i1_.ins.name)
                if j < T:
                    v = wp.tile([P, BL], dt.float32, tag="v")
                    # u = v - 1 = 0.5*cu - qsum
                    i2 = nc.vector.scalar_tensor_tensor(
                        out=v[:, :], in0=cu_prev[:, :], scalar=0.5,
                        in1=ps, op0=OP.mult, op1=OP.subtract,
                    )
                    cu = wp.tile([P, BL], dt.float32, tag="cu")
                    # cu = v*m = (u + 1)*m
                    i3 = nc.vector.scalar_tensor_tensor(
                        out=cu[:, :], in0=v[:, :], scalar=1.0,
                        in1=M_t[:, ts(j + 1, BL)], op0=OP.add, op1=OP.mult,
                    )
                    loop_stt_names.extend([i2.ins.name, i3.ins.name])
                    v_names.append(i2.ins.name)
                    cu_prev = cu
            for jj in sorted(extras):
                for th in extras[jj]:
                    th()

            # ---- tail: last stat blocks, then the output scale ----
            for bk in (249, 250, 251):
                nc.vector.tensor_tensor(
                    out=S_t[H:P, :], in0=S_t[H:P, :],
                    in1=M_t[H:P, ts(bk, BL)], op=OP.add,
                )
            nc.vector.tensor_scalar(
                out=R_t[H:P, :], in0=S_t[H:P, :],
                scalar1=235.0, scalar2=-1.0 / 235.0,
                op0=OP.subtract, op1=OP.mult,
            )
            nc.sync.dma_start(out=out_d[:, :], in_=R_t[H:P, :])

    nc._loop_stt_names = loop_stt_names + stat_add_names
    nc._v_names = v_names
    nc._tail_mm_names = tail_mm_names
    nc._loop_mm_names = loop_mm_names
    nc._big_mm_names = big_mm_names
    _fix_sync(nc)
    return nc


def _eng_of(w):
    n = w.ant_name
    if "DVE" in n:
        return "DVE"
    if "Activation" in n:
        return "ACT"
    if "PE" in n:
        return "PE"
    if "Pool" in n:
        return "POOL"
    return "OTHER"


def _fix_sync(nc):
    """walrus accepts only ONE sync wait per compute instruction (AC/MM/STT).
    Keep, per instruction, the single wait that engine-order transitivity
    cannot cover:

      * loop STTs (mask/v/cu) and stat adds: the mask keeps its PE (psum)
        wait; the others drop their same-engine self-waits (DVE executes in
        order, and consecutive [84,32] ops observe each other's writes at
        issue cadence -- the baseline already relied on this for mask<-cu).
      * loop matmuls: keep the DVE wait (recurrent-mask RAW).  The psum
        accumulation group ordering vs the bigmm matmuls is PE-in-order.
      * bigmm matmuls: keep the DMA (xtile) wait.  The psA-slot WAR vs the
        DVE readers of 4 blocks ago is covered by the kept DVE waits of the
        loop matmuls that precede this matmul in the PE stream (DVE
        semaphore counts are completion-ordered), with ~50 periods margin.
      * tail bias matmul: keep the DVE (ones2 memset) wait; its weight DMA
        is covered by the prologue LDWEIGHTS of the same tile.
      * Drains keep only output-DMA lanes (input-DMA completions are covered
        by their consumers' waits; engine completion by the final barrier).
    """
    import concourse.mybir as mybir

    tail_mm = set(nc._tail_mm_names)
    loop_mm = set(nc._loop_mm_names)
    big_mm = set(nc._big_mm_names)
    loop_stt = set(nc._loop_stt_names)

    out_names = set()
    for alloc in nc.m.functions[0].allocations:
        if (
            isinstance(alloc, mybir.MemoryLocationSet)
            and alloc.kind == "ExternalOutput"
        ):
            for ml in alloc.memorylocations:
                out_names.add(ml.name)
    keep_lanes = set()
    for name, inst in nc.inst_map.items():
        if "DMA" not in type(inst).__name__:
            continue
        c = inst.concise()
        if any(f"@{n}" in c.split("in=")[0] for n in out_names):
            for u in (inst.sync_info.on_update or []) if inst.sync_info else []:
                keep_lanes.add(u.ant_name)

    problems = []
    for name, inst in nc.inst_map.items():
        si = inst.sync_info
        if si is None or not si.on_wait:
            continue
        waits = list(si.on_wait)
        own = {u.ant_name for u in (si.on_update or [])}

        if name in loop_mm or name in tail_mm:
            kept = [w for w in waits if _eng_of(w) == "DVE"]
            if not kept:
                kept = [w for w in waits if w.ant_name not in own]
        elif name in big_mm:
            kept = [w for w in waits
                    if _eng_of(w) not in ("DVE", "ACT")
                    and w.ant_name not in own]
            if not kept and waits:
                kept = [w for w in waits if w.ant_name not in own][:1]
        elif name in loop_stt:
            kept = [w for w in waits if w.ant_name not in own]
        elif len(waits) >= 2:
            kept = [w for w in waits if w.ant_name not in own]
            if "Drain" in type(inst).__name__ and len(kept) > 1:
                kept = [w for w in kept if w.ant_name in keep_lanes]
        else:
            continue

        if len(kept) != len(waits):
            si.on_wait = kept
        if len(kept) > 1 and "Drain" not in type(inst).__name__ \
                and "DMA" not in type(inst).__name__ \
                and "Branch" not in type(inst).__name__:
            problems.append((name, type(inst).__name__,
                             [w.ant_name for w in kept]))
    if problems:
        for p in problems[:8]:
            print("MULTIWAIT:", p)

def _prep_shared(W1, b1, Wr, br, W2, b2):
    f32 = np.float32
    W1 = np.asarray(W1, f32); b1 = np.asarray(b1, f32)
    Wr = np.asarray(Wr, f32); br = np.asarray(br, f32)
    W2 = np.asarray(W2, f32); b2 = np.asarray(b2, f32)
    Wrh, Wry = Wr[:, :H], Wr[:, H:]
    # Negated ("qsum = 1 - v") encoding: psum = (1-bt) - Wtil@x
    # - 0.5*[Wry;W2]@m with m in {0,1}; spike test is then 0.5*cu < q.
    Wtil = -0.5 * (Wrh @ W1)                                  # [64, 700]
    bt1 = 0.5 * (Wrh @ b1 + br + Wry.sum(axis=1))
    bt2 = 0.5 * (b2 + W2.sum(axis=1))
    bfl = 1.0 - np.concatenate([bt1, bt2])                    # [84] fp32
    Wtp = np.zeros((P, DP), f32)
    Wtp[:H, :D] = Wtil
    # Bias rides two spare contraction rows as a double-bf16 split; the
    # matching x rows are constant 1.0, so psum picks up ~fp32 bias.
    bhi = bfl.astype(BF16).astype(f32)
    Wtp[:, BIAS_ROW] = bhi
    Wtp[:, BIAS_ROW + 1] = bfl - bhi
    Wt6 = np.ascontiguousarray(
        Wtp.reshape(P, KCH, 128).transpose(1, 2, 0)
    ).astype(BF16)                                            # [6, 128, 84]
    L = np.concatenate([0.5 * Wry.T, 0.5 * W2.T], axis=1).astype(BF16)
    return Wt6, L


def _core_xt(xc_bf):
    """Build one core's permuted drive matrix [KCH, 128, NCOLP] from its
    (BL, T, D) bf16 input slice: step j lands in column group BLK_S[q]+c
    (the bank-interleaved layout), bias rows are constant 1."""
    xt = np.zeros((DP, NCOLP // BL, BL), BF16)
    gidx = np.empty(TSTEPS, np.int64)
    for j in range(TSTEPS):
        q, c = _slice_of(j)
        gidx[j] = BLK_S[q] + c
    xt[:D, gidx[:T]] = xc_bf.transpose(2, 1, 0)
    xt[BIAS_ROW:BIAS_ROW + 2] = 1.0
    return np.ascontiguousarray(xt.reshape(KCH, 128, NCOLP))


def _ensure_ntff_hook():
    """The RL container's antenv stub lacks axon_hooks; bass_utils imports it
    unconditionally when tracing. Register the ctypes-based hook ourselves."""
    import sys
    import types
    try:
        import antenv
        if "antenv.axon_hooks" in sys.modules:
            return
        mod = types.ModuleType("antenv.axon_hooks")
        _h = [None]
        mod.set_axon_ntff_profile_hook = lambda h: _h.__setitem__(0, h)
        mod.get_axon_ntff_profile_hook = lambda: _h[0]
        sys.modules["antenv.axon_hooks"] = mod
        antenv.axon_hooks = mod
        try:
            from trn_agent_boot.trn_boot import _ntff_profile_via_ctypes
            mod.set_axon_ntff_profile_hook(
                _ntff_profile_via_ctypes("/opt/axon/libaxon_pjrt.so")
            )
        except Exception:
            pass
    except Exception:
        pass


def kernel(x, W1, b1, Wr, br, W2, b2):
    from concourse.bass_utils import run_bass_kernel_spmd

    _ensure_ntff_hook()

    if "nc" not in _CACHE:
        _CACHE["nc"] = _build_nc()
    nc = _CACHE["nc"]

    Wt6, L = _prep_shared(W1, b1, Wr, br, W2, b2)

    x = np.asarray(x, np.float32)
    xbf = x.astype(BF16)                                      # (B, T, D)
    in_maps = []
    for c in range(NCORES):
        xc = xbf[c * BL:(c + 1) * BL]                         # (32, 250, 700)
        in_maps.append({
            "xT": _core_xt(xc), "Wt": Wt6, "Lw": L,
        })

    res = run_bass_kernel_spmd(nc, in_maps, core_ids=list(range(NCORES)))
    _CACHE["last_results"] = res
    out = np.concatenate(
        [np.asarray(r["out"]).T for r in res.results], axis=0
    ).astype(np.float32)                                      # (256, 20)
    return out


# revision 26
# speedup vs baseline: 1.5307x; 1.2525x over previous
"""Trainium2 Bass kernel for nn_CompNet (spiking LIF RNN).

Math summary (reformulation of the reference):
  Per step t:  h = W1 x_t + b1;  i = Wr [h; y] + br
               v1 <- 0.5 v1 + 0.5 i ; s1 = (v1>=1); v1 *= (1-s1)
               logits = W2 s1 + b2
               v2 <- 0.5 v2 + 0.5 logits ; s2 = (v2>=1); v2 *= (1-s2)
  out = mean_{t>=15} s2                                    -> (B, C)

Key algebraic folds (all host-side, exact in fp32):
  * h only enters via Wr_h @ h, so fold:  Wtil = 0.5*Wr_h@W1   (64x700)
  * substitute s = 1 - m with m = (v < 1), folding the constant
    Wr_y@1 / W2@1 terms into per-population biases:
       bt1 = 0.5*(Wr_h b1 + br + Wr_y 1),  bt2 = 0.5*(b2 + W2 1)
  * LIF1 (rows 0..63) and LIF2 (rows 64..83) are stacked into one 84-row
    population, with LIF2 lagging one step (its drive only needs s1 of the
    previous loop iteration).

Per-core execution (feature-major, batch on the free axis, B_local=32):
  bigmm (PE):  psA block b [84,512] = Wt@x for 16 steps (6 matmuls; the
               bias rides two spare contraction rows as a double-bf16
               split against constant-1 rows of x, so psum = drive+bias)
  loop j (PE): psA slice [84,32] += L@Mbuf[0:64, blk j]  (1 matmul, acc)
  loop j (DVE): m*_j = (0.5*cu_{j-1} < psum_j) -> Mbuf blk j+1
                v_j  = 0.5*cu_{j-1} - psum_j
                cu_j = (v_j + 1)*m*_j
                S   += Mbuf[64:84, blk j-2]   (one hidden stat add/iter)
  Output: out = (S - 117.5)*(-2/235)

Sync strategy: walrus accepts ONE wait per compute instruction.  Each
instruction keeps exactly the one wait that is not transitively covered
by engine-order (PE/ACT/DVE streams are in-order); see _fix_sync.

Sharding: pure data parallelism, batch 256 -> 8 cores x 32.
"""

import numpy as np
import ml_dtypes

BF16 = ml_dtypes.bfloat16

B, T, D, H, C = 256, 250, 700, 64, 20
NCORES = 8
BL = B // NCORES          # 32 batch per core
P = H + C                 # 84 stacked feature rows
KCH = 6                   # ceil(700/128) contraction chunks
DP = KCH * 128            # 768 padded feature dim
NCOL = T * BL             # 8000 drive columns per core
BIAS_ROW = 704            # 64-aligned bias rows inside the padded contraction
VTH_INIT = 2.0e9          # suppresses the phantom LIF2 step at j=0
TSTEPS = T + 1            # 251 loop steps incl the bias-only last one
GEN = 64                  # steps per bank generation (4 banks x 16 cols)
NBGEN = 4                 # psum banks per generation (groups alternate)

# Steps are permuted so consecutive iterations hit different psum banks:
# step j = 64g + 4c + r lives in block q=4g+r (bank 4*(g%2)+r), column c.
# That keeps each matmul's bank-WAR 4 iterations stale, so its only live
# dependency is the previous mask (the recurrent-spike RAW).
def _slice_of(j):
    g, o = divmod(j, GEN)
    return NBGEN * g + o % NBGEN, o // NBGEN

NGENS = (TSTEPS + GEN - 1) // GEN                      # 4
NBLK = NBGEN * NGENS                                   # 16 blocks
BLK_W = [0] * NBLK                                     # cols (steps) per block
for _j in range(TSTEPS):
    BLK_W[_slice_of(_j)[0]] += 1
BLK_S = [0] * (NBLK + 1)                               # start col-group
for _q in range(NBLK):
    BLK_S[_q + 1] = BLK_S[_q] + BLK_W[_q]
NCOLP = BLK_S[NBLK] * BL                               # 8032 permuted columns
CH_BLOCKS = [[0], [1, 2, 3], [4, 5, 6, 7], [8, 9, 10, 11], [12, 13, 14, 15]]
CH_COLS = [sum(BLK_W[b] for b in grp) * BL for grp in CH_BLOCKS]

_CACHE = {}
SIM_SAFE_STOPS = False    # True: stop every psum slice (CoreSim read lint)


def _build_nc():
    import concourse.bass as bass
    import concourse.mybir as mybir
    from concourse.tile import TileContext

    dt = mybir.dt
    AF = mybir.ActivationFunctionType
    OP = mybir.AluOpType
    ts = bass.ts

    # detect_race_conditions=False: the hand-managed single-wait sync (see
    # _fix_sync) relies on engine-order transitivity the simulator's race
    # detector cannot see.
    nc = bass.Bass(
        "TRN2", target_bir_lowering=False, debug=False,
        detect_race_conditions=False,
    )

    xT = nc.dram_tensor("xT", [KCH, 128, NCOLP], dt.bfloat16, kind="ExternalInput").ap()
    Wt = nc.dram_tensor("Wt", [KCH, 128, P], dt.bfloat16, kind="ExternalInput").ap()
    Lw = nc.dram_tensor("Lw", [H, P], dt.bfloat16, kind="ExternalInput").ap()
    out_d = nc.dram_tensor("out", [C, BL], dt.float32, kind="ExternalOutput").ap()

    # chunk start columns
    ch_start = [0]
    for w in CH_COLS:
        ch_start.append(ch_start[-1] + w)

    loop_stt_names = []
    v_names = []
    stat_add_names = []
    tail_mm_names = []
    loop_mm_names = []
    big_mm_names = []

    with TileContext(nc) as tc:
        with (
            tc.tile_pool(name="const", bufs=1) as cp,
            tc.tile_pool(name="xs", bufs=3) as xp,
            tc.tile_pool(name="wk", bufs=4) as wp,
            tc.tile_pool(name="psA", bufs=8, space="PSUM") as psA,
        ):
            # ---- persistent tiles ----
            M_t = cp.tile([P, NCOL + 2 * BL], dt.bfloat16, tag="M")    # blocks 0..251
            wts = [
                cp.tile([128, P], dt.bfloat16, tag=f"w{k}", name=f"wts{k}")
                for k in range(KCH)
            ]
            L_t = cp.tile([H, P], dt.bfloat16, tag="L")
            cu0 = cp.tile([P, BL], dt.float32, tag="cu0")
            S_t = cp.tile([P, BL], dt.float32, tag="S")
            R_t = cp.tile([P, BL], dt.float32, tag="R")

            # ---- prologue: weights, inits ----
            for k in range(KCH):
                nc.sync.dma_start(out=wts[k][:, :], in_=Wt[k, :, :])
            nc.sync.dma_start(out=L_t[:, :], in_=Lw[:, :])

            nc.vector.memset(M_t[0:H, 0:BL], 1.0)     # m_{-1} = 1 (y=0)
            nc.vector.memset(M_t[H:P, 0:BL], 0.0)
            nc.vector.memset(cu0[0:H, :], 0.0)        # v1 carry starts at 0
            nc.gpsimd.memset(S_t[H:P, :], 0.0)        # output statistic
            nc.vector.memset(cu0[H:P, :], VTH_INIT)   # kill phantom LIF2 step

            # ---- x DMAs ----
            xtiles = {}

            def emit_xdma(c):
                c0, w = ch_start[c], CH_COLS[c]
                for k in range(KCH):
                    t = xp.tile([128, 2048], dt.bfloat16, tag=f"x{k}")
                    nc.sync.dma_start(out=t[:, 0:w], in_=xT[k, :, c0:c0 + w])
                    xtiles[(c, k)] = t

            # ---- big matmul, 512-col psA blocks ----
            pa_tiles = {}

            def bigmm_block_ops(q):
                """Yield 6 matmul thunks computing psA block q (one bank,
                up to 512 cols = 16 interleaved steps)."""
                col0 = BLK_S[q] * BL
                nw = BLK_W[q] * BL
                c = next(i for i in range(len(CH_BLOCKS))
                         if q in CH_BLOCKS[i])
                n0 = col0 - sum(CH_COLS[:c])
                pa = psA.tile([P, 512], dt.float32, tag="pa")
                pa_tiles[q] = (pa, BLK_W[q])

                def mk_mm(k, pa=pa, n0=n0, nw=nw, c=c):
                    def f():
                        i = nc.tensor.matmul(
                            out=pa[:, 0:nw], lhsT=wts[k][:, :],
                            rhs=xtiles[(c, k)][:, n0:n0 + nw],
                            start=(k == 0), stop=False,
                        )
                        big_mm_names.append(i.ins.name)
                    return f

                for k in range(KCH):
                    yield mk_mm(k)

            # ---- interleave schedule ----
            extras = {}
            for g in range(1, NGENS):
                for r in range(NBGEN):
                    q = NBGEN * g + r
                    for i, th in enumerate(bigmm_block_ops(q)):
                        extras.setdefault(GEN * (g - 1) + 8 + 6 * r + i,
                                          []).append(th)
            extras.setdefault(36, []).append(lambda: emit_xdma(3))
            extras.setdefault(100, []).append(lambda: emit_xdma(4))

            # prologue: x chunks 0..2 + psA blocks 0..3 (generation 0)
            emit_xdma(0)
            emit_xdma(1)
            emit_xdma(2)
            for q in range(NBGEN):
                for th in bigmm_block_ops(q):
                    th()

            # ---- the sequential LIF loop ----
            cu_prev = cu0
            for j in range(TSTEPS):
                for th in extras.pop(j, []):
                    th()
                # stat add on the (otherwise idle) Pool engine: a 4th DVE
                # op per iteration throttles the DVE issue pipe to ~630ns.
                if 19 <= j:
                    bk = j - 2      # stat blocks 17..248 inside the loop
                    i4 = nc.gpsimd.tensor_tensor(
                        out=S_t[H:P, :], in0=S_t[H:P, :],
                        in1=M_t[H:P, ts(bk, BL)], op=OP.add,
                    )
                    stat_add_names.append(i4.ins.name)
                q, c = _slice_of(j)
                pa, wq = pa_tiles[q]
                ps = pa[:, c * BL:(c + 1) * BL]
                # stop on the bank's last slice only (stop flushes the whole
                # bank); CoreSim's read lint wants stop before psum reads,
                # so the sim build stops every slice (no data effect).
                im = nc.tensor.matmul(
                    out=ps, lhsT=L_t[:, :], rhs=M_t[0:H, ts(j, BL)],
                    start=False, stop=SIM_SAFE_STOPS or c == wq - 1,
                    skip_group_check=(c != 0),
                )
                loop_mm_names.append(im.ins.name)
                # qsum = 1 - (drive + recurrent); spike mask straight off
                # PSUM in ONE fused op:  v < 1  <=>  0.5*cu < qsum.
                i1_ = nc.vector.scalar_tensor_tensor(
                    out=M_t[:, ts(j + 1, BL)], in0=cu_prev[:, :], scalar=0.5,
                    in1=ps, op0=OP.mult, op1=OP.is_lt,
                )
                loop_stt_names.append(i1_.ins.name)
                if j < T:
                    v = wp.tile([P, BL], dt.float32, tag="v")
                    # u = v - 1 = 0.5*cu - qsum
                    i2 = nc.vector.scalar_tensor_tensor(
                        out=v[:, :], in0=cu_prev[:, :], scalar=0.5,
                        in1=ps, op0=OP.mult, op1=OP.subtract,
                    )
                    cu = wp.tile([P, BL], dt.float32, tag="cu")
                    # cu = v*m = (u + 1)*m
                    i3 = nc.vector.scalar_tensor_tensor(
                        out=cu[:, :], in0=v[:, :], scalar=1.0,
                        in1=M_t[:, ts(j + 1, BL)], op0=OP.add, op1=OP.mult,
                    )
                    loop_stt_names.extend([i2.ins.name, i3.ins.name])
                    v_names.append(i2.ins.name)
                    cu_prev = cu
            for jj in sorted(extras):
                for th in extras[jj]:
                    th()

            # ---- tail: last stat blocks, then the output scale ----
            for bk in (249, 250, 251):
                nc.gpsimd.tensor_tensor(
                    out=S_t[H:P, :], in0=S_t[H:P, :],
                    in1=M_t[H:P, ts(bk, BL)], op=OP.add,
                )
            nc.gpsimd.tensor_scalar(
                out=R_t[H:P, :], in0=S_t[H:P, :],
                scalar1=235.0, scalar2=-1.0 / 235.0,
                op0=OP.subtract, op1=OP.mult,
            )
            nc.sync.dma_start(out=out_d[:, :], in_=R_t[H:P, :])

    nc._loop_stt_names = loop_stt_names + stat_add_names
    nc._v_names = v_names
    nc._tail_mm_names = tail_mm_names
    nc._loop_mm_names = loop_mm_names
    nc._big_mm_names = big_mm_names
    _fix_sync(nc)
    return nc


def _eng_of(w):
    n = w.ant_name
    if "DVE" in n:
        return "DVE"
    if "Activation" in n:
        return "ACT"
    if "PE" in n:
        return "PE"
    if "Pool" in n:
        return "POOL"
    return "OTHER"


def _fix_sync(nc):
    """walrus accepts only ONE sync wait per compute instruction (AC/MM/STT).
    Keep, per instruction, the single wait that engine-order transitivity
    cannot cover:

      * loop STTs (mask/v/cu) and stat adds: the mask keeps its PE (psum)
        wait; the others drop their same-engine self-waits (DVE executes in
        order, and consecutive [84,32] ops observe each other's writes at
        issue cadence -- the baseline already relied on this for mask<-cu).
      * loop matmuls: keep the DVE wait (recurrent-mask RAW).  The psum
        accumulation group ordering vs the bigmm matmuls is PE-in-order.
      * bigmm matmuls: keep the DMA (xtile) wait.  The psA-slot WAR vs the
        DVE readers of 4 blocks ago is covered by the kept DVE waits of the
        loop matmuls that precede this matmul in the PE stream (DVE
        semaphore counts are completion-ordered), with ~50 periods margin.
      * tail bias matmul: keep the DVE (ones2 memset) wait; its weight DMA
        is covered by the prologue LDWEIGHTS of the same tile.
      * Drains keep only output-DMA lanes (input-DMA completions are covered
        by their consumers' waits; engine completion by the final barrier).
    """
    import concourse.mybir as mybir

    tail_mm = set(nc._tail_mm_names)
    loop_mm = set(nc._loop_mm_names)
    big_mm = set(nc._big_mm_names)
    loop_stt = set(nc._loop_stt_names)

    out_names = set()
    for alloc in nc.m.functions[0].allocations:
        if (
            isinstance(alloc, mybir.MemoryLocationSet)
            and alloc.kind == "ExternalOutput"
        ):
            for ml in alloc.memorylocations:
                out_names.add(ml.name)
    keep_lanes = set()
    for name, inst in nc.inst_map.items():
        if "DMA" not in type(inst).__name__:
            continue
        c = inst.concise()
        if any(f"@{n}" in c.split("in=")[0] for n in out_names):
            for u in (inst.sync_info.on_update or []) if inst.sync_info else []:
                keep_lanes.add(u.ant_name)

    problems = []
    for name, inst in nc.inst_map.items():
        si = inst.sync_info
        if si is None or not si.on_wait:
            continue
        waits = list(si.on_wait)
        own = {u.ant_name for u in (si.on_update or [])}

        if name in loop_mm or name in tail_mm:
            kept = [w for w in waits if _eng_of(w) == "DVE"]
            if not kept:
                kept = [w for w in waits if w.ant_name not in own]
        elif name in big_mm:
            kept = [w for w in waits
                    if _eng_of(w) not in ("DVE", "ACT")
                    and w.ant_name not in own]
            if not kept and waits:
                kept = [w for w in waits if w.ant_name not in own][:1]
        elif name in loop_stt:
            kept = [w for w in waits if w.ant_name not in own]
        elif len(waits) >= 2:
            kept = [w for w in waits if w.ant_name not in own]
            if "Drain" in type(inst).__name__ and len(kept) > 1:
                kept = [w for w in kept if w.ant_name in keep_lanes]
        else:
            continue

        if len(kept) != len(waits):
            si.on_wait = kept
        if len(kept) > 1 and "Drain" not in type(inst).__name__ \
                and "DMA" not in type(inst).__name__ \
                and "Branch" not in type(inst).__name__:
            problems.append((name, type(inst).__name__,
                             [w.ant_name for w in kept]))
    if problems:
        for p in problems[:8]:
            print("MULTIWAIT:", p)

def _prep_shared(W1, b1, Wr, br, W2, b2):
    f32 = np.float32
    W1 = np.asarray(W1, f32); b1 = np.asarray(b1, f32)
    Wr = np.asarray(Wr, f32); br = np.asarray(br, f32)
    W2 = np.asarray(W2, f32); b2 = np.asarray(b2, f32)
    Wrh, Wry = Wr[:, :H], Wr[:, H:]
    # Negated ("qsum = 1 - v") encoding: psum = (1-bt) - Wtil@x
    # - 0.5*[Wry;W2]@m with m in {0,1}; spike test is then 0.5*cu < q.
    Wtil = -0.5 * (Wrh @ W1)                                  # [64, 700]
    bt1 = 0.5 * (Wrh @ b1 + br + Wry.sum(axis=1))
    bt2 = 0.5 * (b2 + W2.sum(axis=1))
    bfl = 1.0 - np.concatenate([bt1, bt2])                    # [84] fp32
    Wtp = np.zeros((P, DP), f32)
    Wtp[:H, :D] = Wtil
    # Bias rides two spare contraction rows as a double-bf16 split; the
    # matching x rows are constant 1.0, so psum picks up ~fp32 bias.
    bhi = bfl.astype(BF16).astype(f32)
    Wtp[:, BIAS_ROW] = bhi
    Wtp[:, BIAS_ROW + 1] = bfl - bhi
    Wt6 = np.ascontiguousarray(
        Wtp.reshape(P, KCH, 128).transpose(1, 2, 0)
    ).astype(BF16)                                            # [6, 128, 84]
    L = np.concatenate([0.5 * Wry.T, 0.5 * W2.T], axis=1).astype(BF16)
    return Wt6, L


def _core_xt(xc_bf):
    """Build one core's permuted drive matrix [KCH, 128, NCOLP] from its
    (BL, T, D) bf16 input slice: step j lands in column group BLK_S[q]+c
    (the bank-interleaved layout), bias rows are constant 1."""
    xt = np.zeros((DP, NCOLP // BL, BL), BF16)
    gidx = np.empty(TSTEPS, np.int64)
    for j in range(TSTEPS):
        q, c = _slice_of(j)
        gidx[j] = BLK_S[q] + c
    xt[:D, gidx[:T]] = xc_bf.transpose(2, 1, 0)
    xt[BIAS_ROW:BIAS_ROW + 2] = 1.0
    return np.ascontiguousarray(xt.reshape(KCH, 128, NCOLP))


def _ensure_ntff_hook():
    """The RL container's antenv stub lacks axon_hooks; bass_utils imports it
    unconditionally when tracing. Register the ctypes-based hook ourselves."""
    import sys
    import types
    try:
        import antenv
        if "antenv.axon_hooks" in sys.modules:
            return
        mod = types.ModuleType("antenv.axon_hooks")
        _h = [None]
        mod.set_axon_ntff_profile_hook = lambda h: _h.__setitem__(0, h)
        mod.get_axon_ntff_profile_hook = lambda: _h[0]
        sys.modules["antenv.axon_hooks"] = mod
        antenv.axon_hooks = mod
        try:
            from trn_agent_boot.trn_boot import _ntff_profile_via_ctypes
            mod.set_axon_ntff_profile_hook(
                _ntff_profile_via_ctypes("/opt/axon/libaxon_pjrt.so")
            )
        except Exception:
            pass
    except Exception:
        pass


def kernel(x, W1, b1, Wr, br, W2, b2):
    from concourse.bass_utils import run_bass_kernel_spmd

    _ensure_ntff_hook()

    if "nc" not in _CACHE:
        _CACHE["nc"] = _build_nc()
    nc = _CACHE["nc"]

    Wt6, L = _prep_shared(W1, b1, Wr, br, W2, b2)

    x = np.asarray(x, np.float32)
    xbf = x.astype(BF16)                                      # (B, T, D)
    in_maps = []
    for c in range(NCORES):
        xc = xbf[c * BL:(c + 1) * BL]                         # (32, 250, 700)
        in_maps.append({
            "xT": _core_xt(xc), "Wt": Wt6, "Lw": L,
        })

    res = run_bass_kernel_spmd(nc, in_maps, core_ids=list(range(NCORES)))
    _CACHE["last_results"] = res
    out = np.concatenate(
        [np.asarray(r["out"]).T for r in res.results], axis=0
    ).astype(np.float32)                                      # (256, 20)
    return out


# revision 29
# speedup vs baseline: 1.5418x; 1.0072x over previous
"""Trainium2 Bass kernel for nn_CompNet (spiking LIF RNN).

Math summary (reformulation of the reference):
  Per step t:  h = W1 x_t + b1;  i = Wr [h; y] + br
               v1 <- 0.5 v1 + 0.5 i ; s1 = (v1>=1); v1 *= (1-s1)
               logits = W2 s1 + b2
               v2 <- 0.5 v2 + 0.5 logits ; s2 = (v2>=1); v2 *= (1-s2)
  out = mean_{t>=15} s2                                    -> (B, C)

Key algebraic folds (all host-side, exact in fp32):
  * h only enters via Wr_h @ h, so fold:  Wtil = 0.5*Wr_h@W1   (64x700)
  * substitute s = 1 - m with m = (v < 1), folding the constant
    Wr_y@1 / W2@1 terms into per-population biases:
       bt1 = 0.5*(Wr_h b1 + br + Wr_y 1),  bt2 = 0.5*(b2 + W2 1)
  * LIF1 (rows 0..63) and LIF2 (rows 64..83) are stacked into one 84-row
    population, with LIF2 lagging one step (its drive only needs s1 of the
    previous loop iteration).

Per-core execution (feature-major, batch on the free axis, B_local=32):
  bigmm (PE):  psA block b [84,512] = Wt@x for 16 steps (6 matmuls; the
               bias rides two spare contraction rows as a double-bf16
               split against constant-1 rows of x, so psum = drive+bias)
  loop j (PE): psA slice [84,32] += L@Mbuf[0:64, blk j]  (1 matmul, acc)
  loop j (DVE): m*_j = (0.5*cu_{j-1} < psum_j) -> Mbuf blk j+1
                v_j  = 0.5*cu_{j-1} - psum_j
                cu_j = (v_j + 1)*m*_j
                S   += Mbuf[64:84, blk j-2]   (one hidden stat add/iter)
  Output: out = (S - 117.5)*(-2/235)

Sync strategy: walrus accepts ONE wait per compute instruction.  Each
instruction keeps exactly the one wait that is not transitively covered
by engine-order (PE/ACT/DVE streams are in-order); see _fix_sync.

Sharding: pure data parallelism, batch 256 -> 8 cores x 32.
"""

import numpy as np
import ml_dtypes

BF16 = ml_dtypes.bfloat16

B, T, D, H, C = 256, 250, 700, 64, 20
NCORES = 8
BL = B // NCORES          # 32 batch per core
P = H + C                 # 84 stacked feature rows
KCH = 6                   # ceil(700/128) contraction chunks
DP = KCH * 128            # 768 padded feature dim
NCOL = T * BL             # 8000 drive columns per core
BIAS_ROW = 704            # 64-aligned bias rows inside the padded contraction
VTH_INIT = 2.0e9          # suppresses the phantom LIF2 step at j=0
TSTEPS = T + 1            # 251 loop steps incl the bias-only last one
GEN = 64                  # steps per bank generation (4 banks x 16 cols)
NBGEN = 4                 # psum banks per generation (groups alternate)

# Steps are permuted so consecutive iterations hit different psum banks:
# step j = 64g + 4c + r lives in block q=4g+r (bank 4*(g%2)+r), column c.
# That keeps each matmul's bank-WAR 4 iterations stale, so its only live
# dependency is the previous mask (the recurrent-spike RAW).
def _slice_of(j):
    g, o = divmod(j, GEN)
    return NBGEN * g + o % NBGEN, o // NBGEN

NGENS = (TSTEPS + GEN - 1) // GEN                      # 4
NBLK = NBGEN * NGENS                                   # 16 blocks
BLK_W = [0] * NBLK                                     # cols (steps) per block
for _j in range(TSTEPS):
    BLK_W[_slice_of(_j)[0]] += 1
BLK_S = [0] * (NBLK + 1)                               # start col-group
for _q in range(NBLK):
    BLK_S[_q + 1] = BLK_S[_q] + BLK_W[_q]
NCOLP = BLK_S[NBLK] * BL                               # 8032 permuted columns
CH_BLOCKS = [[0], [1], [2], [3], [4, 5], [6, 7],
             [8, 9], [10, 11], [12, 13], [14, 15]]
CH_COLS = [sum(BLK_W[b] for b in grp) * BL for grp in CH_BLOCKS]

_CACHE = {}
SIM_SAFE_STOPS = False    # True: stop every psum slice (CoreSim read lint)


def _build_nc():
    import concourse.bass as bass
    import concourse.mybir as mybir
    from concourse.tile import TileContext

    dt = mybir.dt
    AF = mybir.ActivationFunctionType
    OP = mybir.AluOpType
    ts = bass.ts

    # detect_race_conditions=False: the hand-managed single-wait sync (see
    # _fix_sync) relies on engine-order transitivity the simulator's race
    # detector cannot see.
    nc = bass.Bass(
        "TRN2", target_bir_lowering=False, debug=False,
        detect_race_conditions=False,
    )

    xT = nc.dram_tensor("xT", [KCH, 128, NCOLP], dt.bfloat16, kind="ExternalInput").ap()
    Wt = nc.dram_tensor("Wt", [KCH, 128, P], dt.bfloat16, kind="ExternalInput").ap()
    Lw = nc.dram_tensor("Lw", [H, P], dt.bfloat16, kind="ExternalInput").ap()
    out_d = nc.dram_tensor("out", [C, BL], dt.float32, kind="ExternalOutput").ap()

    # chunk start columns
    ch_start = [0]
    for w in CH_COLS:
        ch_start.append(ch_start[-1] + w)

    loop_stt_names = []
    v_names = []
    stat_add_names = []
    tail_mm_names = []
    loop_mm_names = []
    big_mm_names = []

    with TileContext(nc) as tc:
        with (
            tc.tile_pool(name="const", bufs=1) as cp,
            tc.tile_pool(name="xs", bufs=5) as xp,
            tc.tile_pool(name="wk", bufs=4) as wp,
            tc.tile_pool(name="psA", bufs=8, space="PSUM") as psA,
        ):
            # ---- persistent tiles ----
            M_t = cp.tile([P, NCOL + 2 * BL], dt.bfloat16, tag="M")    # blocks 0..251
            W_t = cp.tile([128, KCH * P], dt.bfloat16, tag="W")
            L_t = cp.tile([H, P], dt.bfloat16, tag="L")
            cu0 = cp.tile([P, BL], dt.float32, tag="cu0")
            S_t = cp.tile([P, BL], dt.float32, tag="S")
            R_t = cp.tile([P, BL], dt.float32, tag="R")

            # ---- prologue: weights, inits ----
            nc.sync.dma_start(
                out=W_t[:, :],
                in_=bass.AP(Wt.tensor, 0,
                            [[P, 128], [128 * P, KCH], [1, P]]),
            )
            nc.sync.dma_start(out=L_t[:, :], in_=Lw[:, :])

            nc.vector.memset(M_t[0:H, 0:BL], 1.0)     # m_{-1} = 1 (y=0)
            nc.vector.memset(M_t[H:P, 0:BL], 0.0)
            nc.vector.memset(cu0[0:H, :], 0.0)        # v1 carry starts at 0
            nc.gpsimd.memset(S_t[H:P, :], 0.0)        # output statistic
            nc.vector.memset(cu0[H:P, :], VTH_INIT)   # kill phantom LIF2 step

            # ---- x DMAs: one k-major tile and one descriptor per chunk
            # (each dma_start costs ~730ns on the serial SP queue) ----
            xtiles = {}

            def emit_xdma(c):
                c0, w = ch_start[c], CH_COLS[c]
                t = xp.tile([128, KCH * 1024], dt.bfloat16, tag="xc")
                nc.sync.dma_start(
                    out=t[:, 0:KCH * w],
                    in_=bass.AP(xT.tensor, c0,
                                [[NCOLP, 128], [128 * NCOLP, KCH], [1, w]]),
                )
                xtiles[c] = (t, w)

            # ---- big matmul, 512-col psA blocks ----
            pa_tiles = {}

            def bigmm_block_ops(q):
                """Yield 6 matmul thunks computing psA block q (one bank,
                up to 512 cols = 16 interleaved steps)."""
                col0 = BLK_S[q] * BL
                nw = BLK_W[q] * BL
                c = next(i for i in range(len(CH_BLOCKS))
                         if q in CH_BLOCKS[i])
                n0 = col0 - sum(CH_COLS[:c])
                pa = psA.tile([P, 512], dt.float32, tag="pa")
                pa_tiles[q] = (pa, BLK_W[q])

                def mk_mm(k, pa=pa, n0=n0, nw=nw, c=c):
                    def f():
                        t, w = xtiles[c]
                        i = nc.tensor.matmul(
                            out=pa[:, 0:nw],
                            lhsT=W_t[:, k * P:(k + 1) * P],
                            rhs=t[:, k * w + n0:k * w + n0 + nw],
                            start=(k == 0), stop=False,
                        )
                        big_mm_names.append(i.ins.name)
                    return f

                for k in range(KCH):
                    yield mk_mm(k)

            # ---- interleave schedule ----
            extras = {}
            for g in range(1, NGENS):
                for r in range(NBGEN):
                    q = NBGEN * g + r
                    for i, th in enumerate(bigmm_block_ops(q)):
                        extras.setdefault(GEN * g - 24 + 6 * r + i,
                                          []).append(th)
            for c in range(4, len(CH_BLOCKS)):
                extras.setdefault(c - 3, []).append(lambda c=c: emit_xdma(c))

            # prologue: x chunks 0..3 + psA blocks 0..3 (generation 0)
            for c in range(4):
                emit_xdma(c)
            for q in range(NBGEN):
                for th in bigmm_block_ops(q):
                    th()

            # ---- the sequential LIF loop ----
            cu_prev = cu0
            for j in range(TSTEPS):
                for th in extras.pop(j, []):
                    th()
                # stat add on the (otherwise idle) Pool engine: a 4th DVE
                # op per iteration throttles the DVE issue pipe to ~630ns.
                if 19 <= j:
                    bk = j - 2      # stat blocks 17..248 inside the loop
                    i4 = nc.gpsimd.tensor_tensor(
                        out=S_t[H:P, :], in0=S_t[H:P, :],
                        in1=M_t[H:P, ts(bk, BL)], op=OP.add,
                    )
                    stat_add_names.append(i4.ins.name)
                q, c = _slice_of(j)
                pa, wq = pa_tiles[q]
                ps = pa[:, c * BL:(c + 1) * BL]
                # stop on the bank's last slice only (stop flushes the whole
                # bank); CoreSim's read lint wants stop before psum reads,
                # so the sim build stops every slice (no data effect).
                im = nc.tensor.matmul(
                    out=ps, lhsT=L_t[:, :], rhs=M_t[0:H, ts(j, BL)],
                    start=False, stop=SIM_SAFE_STOPS or c == wq - 1,
                    skip_group_check=(c != 0),
                )
                loop_mm_names.append(im.ins.name)
                # qsum = 1 - (drive + recurrent); spike mask straight off
                # PSUM in ONE fused op:  v < 1  <=>  0.5*cu < qsum.
                i1_ = nc.vector.scalar_tensor_tensor(
                    out=M_t[:, ts(j + 1, BL)], in0=cu_prev[:, :], scalar=0.5,
                    in1=ps, op0=OP.mult, op1=OP.is_lt,
                )
                loop_stt_names.append(i1_.ins.name)
                if j < T:
                    v = wp.tile([P, BL], dt.float32, tag="v")
                    # u = v - 1 = 0.5*cu - qsum
                    i2 = nc.vector.scalar_tensor_tensor(
                        out=v[:, :], in0=cu_prev[:, :], scalar=0.5,
                        in1=ps, op0=OP.mult, op1=OP.subtract,
                    )
                    cu = wp.tile([P, BL], dt.float32, tag="cu")
                    # cu = v*m = (u + 1)*m
                    i3 = nc.vector.scalar_tensor_tensor(
                        out=cu[:, :], in0=v[:, :], scalar=1.0,
                        in1=M_t[:, ts(j + 1, BL)], op0=OP.add, op1=OP.mult,
                    )
                    loop_stt_names.extend([i2.ins.name, i3.ins.name])
                    v_names.append(i2.ins.name)
                    cu_prev = cu
            for jj in sorted(extras):
                for th in extras[jj]:
                    th()

            # ---- tail: last stat blocks, then the output scale ----
            for bk in (249, 250, 251):
                nc.gpsimd.tensor_tensor(
                    out=S_t[H:P, :], in0=S_t[H:P, :],
                    in1=M_t[H:P, ts(bk, BL)], op=OP.add,
                )
            nc.gpsimd.tensor_scalar(
                out=R_t[H:P, :], in0=S_t[H:P, :],
                scalar1=235.0, scalar2=-1.0 / 235.0,
                op0=OP.subtract, op1=OP.mult,
            )
            nc.sync.dma_start(out=out_d[:, :], in_=R_t[H:P, :])

    nc._loop_stt_names = loop_stt_names + stat_add_names
    nc._v_names = v_names
    nc._tail_mm_names = tail_mm_names
    nc._loop_mm_names = loop_mm_names
    nc._big_mm_names = big_mm_names
    _fix_sync(nc)
    return nc


def _eng_of(w):
    n = w.ant_name
    if "DVE" in n:
        return "DVE"
    if "Activation" in n:
        return "ACT"
    if "PE" in n:
        return "PE"
    if "Pool" in n:
        return "POOL"
    return "OTHER"


def _fix_sync(nc):
    """walrus accepts only ONE sync wait per compute instruction (AC/MM/STT).
    Keep, per instruction, the single wait that engine-order transitivity
    cannot cover:

      * loop STTs (mask/v/cu) and stat adds: the mask keeps its PE (psum)
        wait; the others drop their same-engine self-waits (DVE executes in
        order, and consecutive [84,32] ops observe each other's writes at
        issue cadence -- the baseline already relied on this for mask<-cu).
      * loop matmuls: keep the DVE wait (recurrent-mask RAW).  The psum
        accumulation group ordering vs the bigmm matmuls is PE-in-order.
      * bigmm matmuls: keep the DMA (xtile) wait.  The psA-slot WAR vs the
        DVE readers of 4 blocks ago is covered by the kept DVE waits of the
        loop matmuls that precede this matmul in the PE stream (DVE
        semaphore counts are completion-ordered), with ~50 periods margin.
      * tail bias matmul: keep the DVE (ones2 memset) wait; its weight DMA
        is covered by the prologue LDWEIGHTS of the same tile.
      * Drains keep only output-DMA lanes (input-DMA completions are covered
        by their consumers' waits; engine completion by the final barrier).
    """
    import concourse.mybir as mybir

    tail_mm = set(nc._tail_mm_names)
    loop_mm = set(nc._loop_mm_names)
    big_mm = set(nc._big_mm_names)
    loop_stt = set(nc._loop_stt_names)

    out_names = set()
    for alloc in nc.m.functions[0].allocations:
        if (
            isinstance(alloc, mybir.MemoryLocationSet)
            and alloc.kind == "ExternalOutput"
        ):
            for ml in alloc.memorylocations:
                out_names.add(ml.name)
    keep_lanes = set()
    for name, inst in nc.inst_map.items():
        if "DMA" not in type(inst).__name__:
            continue
        c = inst.concise()
        if any(f"@{n}" in c.split("in=")[0] for n in out_names):
            for u in (inst.sync_info.on_update or []) if inst.sync_info else []:
                keep_lanes.add(u.ant_name)

    problems = []
    for name, inst in nc.inst_map.items():
        si = inst.sync_info
        if si is None or not si.on_wait:
            continue
        waits = list(si.on_wait)
        own = {u.ant_name for u in (si.on_update or [])}

        if name in loop_mm or name in tail_mm:
            kept = [w for w in waits if _eng_of(w) == "DVE"]
            if not kept:
                kept = [w for w in waits if w.ant_name not in own]
        elif name in big_mm:
            kept = [w for w in waits
                    if _eng_of(w) not in ("DVE", "ACT")
                    and w.ant_name not in own]
            if not kept and waits:
                kept = [w for w in waits if w.ant_name not in own][:1]
        elif name in loop_stt:
            kept = [w for w in waits if w.ant_name not in own]
        elif len(waits) >= 2:
            kept = [w for w in waits if w.ant_name not in own]
            if "Drain" in type(inst).__name__ and len(kept) > 1:
                kept = [w for w in kept if w.ant_name in keep_lanes]
            if "DMACopy" in type(inst).__name__ and len(kept) > 1:
                # DMA_DIRECT2D takes a single wait.  The PE wait (WAR vs the
                # slot's previous readers) subsumes the DMA-lane WAW: those
                # readers only executed after the previous transfer landed.
                pe = [w for w in kept if _eng_of(w) == "PE"]
                kept = pe if pe else kept[:1]
        else:
            continue

        if len(kept) != len(waits):
            si.on_wait = kept
        if len(kept) > 1 and "Drain" not in type(inst).__name__ \
                and "DMA" not in type(inst).__name__ \
                and "Branch" not in type(inst).__name__:
            problems.append((name, type(inst).__name__,
                             [w.ant_name for w in kept]))
    if problems:
        for p in problems[:8]:
            print("MULTIWAIT:", p)

def _prep_shared(W1, b1, Wr, br, W2, b2):
    f32 = np.float32
    W1 = np.asarray(W1, f32); b1 = np.asarray(b1, f32)
    Wr = np.asarray(Wr, f32); br = np.asarray(br, f32)
    W2 = np.asarray(W2, f32); b2 = np.asarray(b2, f32)
    Wrh, Wry = Wr[:, :H], Wr[:, H:]
    # Negated ("qsum = 1 - v") encoding: psum = (1-bt) - Wtil@x
    # - 0.5*[Wry;W2]@m with m in {0,1}; spike test is then 0.5*cu < q.
    Wtil = -0.5 * (Wrh @ W1)                                  # [64, 700]
    bt1 = 0.5 * (Wrh @ b1 + br + Wry.sum(axis=1))
    bt2 = 0.5 * (b2 + W2.sum(axis=1))
    bfl = 1.0 - np.concatenate([bt1, bt2])                    # [84] fp32
    Wtp = np.zeros((P, DP), f32)
    Wtp[:H, :D] = Wtil
    # Bias rides two spare contraction rows as a double-bf16 split; the
    # matching x rows are constant 1.0, so psum picks up ~fp32 bias.
    bhi = bfl.astype(BF16).astype(f32)
    Wtp[:, BIAS_ROW] = bhi
    Wtp[:, BIAS_ROW + 1] = bfl - bhi
    Wt6 = np.ascontiguousarray(
        Wtp.reshape(P, KCH, 128).transpose(1, 2, 0)
    ).astype(BF16)                                            # [6, 128, 84]
    L = np.concatenate([0.5 * Wry.T, 0.5 * W2.T], axis=1).astype(BF16)
    return Wt6, L


def _core_xt(xc_bf):
    """Build one core's permuted drive matrix [KCH, 128, NCOLP] from its
    (BL, T, D) bf16 input slice: step j lands in column group BLK_S[q]+c
    (the bank-interleaved layout), bias rows are constant 1."""
    xt = np.zeros((DP, NCOLP // BL, BL), BF16)
    gidx = np.empty(TSTEPS, np.int64)
    for j in range(TSTEPS):
        q, c = _slice_of(j)
        gidx[j] = BLK_S[q] + c
    xt[:D, gidx[:T]] = xc_bf.transpose(2, 1, 0)
    xt[BIAS_ROW:BIAS_ROW + 2] = 1.0
    return np.ascontiguousarray(xt.reshape(KCH, 128, NCOLP))


def _ensure_ntff_hook():
    """The RL container's antenv stub lacks axon_hooks; bass_utils imports it
    unconditionally when tracing. Register the ctypes-based hook ourselves."""
    import sys
    import types
    try:
        import antenv
        if "antenv.axon_hooks" in sys.modules:
            return
        mod = types.ModuleType("antenv.axon_hooks")
        _h = [None]
        mod.set_axon_ntff_profile_hook = lambda h: _h.__setitem__(0, h)
        mod.get_axon_ntff_profile_hook = lambda: _h[0]
        sys.modules["antenv.axon_hooks"] = mod
        antenv.axon_hooks = mod
        try:
            from trn_agent_boot.trn_boot import _ntff_profile_via_ctypes
            mod.set_axon_ntff_profile_hook(
                _ntff_profile_via_ctypes("/opt/axon/libaxon_pjrt.so")
            )
        except Exception:
            pass
    except Exception:
        pass


def kernel(x, W1, b1, Wr, br, W2, b2):
    from concourse.bass_utils import run_bass_kernel_spmd

    _ensure_ntff_hook()

    if "nc" not in _CACHE:
        _CACHE["nc"] = _build_nc()
    nc = _CACHE["nc"]

    Wt6, L = _prep_shared(W1, b1, Wr, br, W2, b2)

    x = np.asarray(x, np.float32)
    xbf = x.astype(BF16)                                      # (B, T, D)
    in_maps = []
    for c in range(NCORES):
        xc = xbf[c * BL:(c + 1) * BL]                         # (32, 250, 700)
        in_maps.append({
            "xT": _core_xt(xc), "Wt": Wt6, "Lw": L,
        })

    res = run_bass_kernel_spmd(nc, in_maps, core_ids=list(range(NCORES)))
    _CACHE["last_results"] = res
    out = np.concatenate(
        [np.asarray(r["out"]).T for r in res.results], axis=0
    ).astype(np.float32)                                      # (256, 20)
    return out


# revision 30
# speedup vs baseline: 1.5726x; 1.0200x over previous
"""Trainium2 Bass kernel for nn_CompNet (spiking LIF RNN).

Math summary (reformulation of the reference):
  Per step t:  h = W1 x_t + b1;  i = Wr [h; y] + br
               v1 <- 0.5 v1 + 0.5 i ; s1 = (v1>=1); v1 *= (1-s1)
               logits = W2 s1 + b2
               v2 <- 0.5 v2 + 0.5 logits ; s2 = (v2>=1); v2 *= (1-s2)
  out = mean_{t>=15} s2                                    -> (B, C)

Key algebraic folds (all host-side, exact in fp32):
  * h only enters via Wr_h @ h, so fold:  Wtil = 0.5*Wr_h@W1   (64x700)
  * substitute s = 1 - m with m = (v < 1), folding the constant
    Wr_y@1 / W2@1 terms into per-population biases:
       bt1 = 0.5*(Wr_h b1 + br + Wr_y 1),  bt2 = 0.5*(b2 + W2 1)
  * LIF1 (rows 0..63) and LIF2 (rows 64..83) are stacked into one 84-row
    population, with LIF2 lagging one step (its drive only needs s1 of the
    previous loop iteration).

Per-core execution (feature-major, batch on the free axis, B_local=32):
  bigmm (PE):  psA block b [84,512] = Wt@x for 16 steps (6 matmuls; the
               bias rides two spare contraction rows as a double-bf16
               split against constant-1 rows of x, so psum = drive+bias)
  loop j (PE): psA slice [84,32] += L@Mbuf[0:64, blk j]  (1 matmul, acc)
  loop j (DVE): m*_j = (0.5*cu_{j-1} < psum_j) -> Mbuf blk j+1
                v_j  = 0.5*cu_{j-1} - psum_j
                cu_j = (v_j + 1)*m*_j
                S   += Mbuf[64:84, blk j-2]   (one hidden stat add/iter)
  Output: out = (S - 117.5)*(-2/235)

Sync strategy: walrus accepts ONE wait per compute instruction.  Each
instruction keeps exactly the one wait that is not transitively covered
by engine-order (PE/ACT/DVE streams are in-order); see _fix_sync.

Sharding: pure data parallelism, batch 256 -> 8 cores x 32.
"""

import numpy as np
import ml_dtypes

BF16 = ml_dtypes.bfloat16

B, T, D, H, C = 256, 250, 700, 64, 20
NCORES = 8
BL = B // NCORES          # 32 batch per core
P = H + C                 # 84 stacked feature rows
KCH = 6                   # ceil(700/128) contraction chunks
DP = KCH * 128            # 768 padded feature dim
NCOL = T * BL             # 8000 drive columns per core
BIAS_ROW = 704            # 64-aligned bias rows inside the padded contraction
VTH_INIT = 2.0e9          # suppresses the phantom LIF2 step at j=0
TSTEPS = T + 1            # 251 loop steps incl the bias-only last one
GEN = 64                  # steps per bank generation (4 banks x 16 cols)
NBGEN = 4                 # psum banks per generation (groups alternate)

# Steps are permuted so consecutive iterations hit different psum banks:
# step j = 64g + 4c + r lives in block q=4g+r (bank 4*(g%2)+r), column c.
# That keeps each matmul's bank-WAR 4 iterations stale, so its only live
# dependency is the previous mask (the recurrent-spike RAW).
def _slice_of(j):
    g, o = divmod(j, GEN)
    return NBGEN * g + o % NBGEN, o // NBGEN

NGENS = (TSTEPS + GEN - 1) // GEN                      # 4
NBLK = NBGEN * NGENS                                   # 16 blocks
BLK_W = [0] * NBLK                                     # cols (steps) per block
for _j in range(TSTEPS):
    BLK_W[_slice_of(_j)[0]] += 1
BLK_S = [0] * (NBLK + 1)                               # start col-group
for _q in range(NBLK):
    BLK_S[_q + 1] = BLK_S[_q] + BLK_W[_q]
NCOLP = BLK_S[NBLK] * BL                               # 8032 permuted columns
CH_BLOCKS = [[0], [1], [2], [3], [4, 5], [6, 7],
             [8, 9], [10, 11], [12, 13], [14, 15]]
CH_COLS = [sum(BLK_W[b] for b in grp) * BL for grp in CH_BLOCKS]

_CACHE = {}
SIM_SAFE_STOPS = False    # True: stop every psum slice (CoreSim read lint)


def _build_nc():
    import concourse.bass as bass
    import concourse.mybir as mybir
    from concourse.tile import TileContext

    dt = mybir.dt
    AF = mybir.ActivationFunctionType
    OP = mybir.AluOpType
    ts = bass.ts

    # detect_race_conditions=False: the hand-managed single-wait sync (see
    # _fix_sync) relies on engine-order transitivity the simulator's race
    # detector cannot see.
    nc = bass.Bass(
        "TRN2", target_bir_lowering=False, debug=False,
        detect_race_conditions=False,
    )

    xT = nc.dram_tensor("xT", [KCH, 128, NCOLP], dt.bfloat16, kind="ExternalInput").ap()
    Wt = nc.dram_tensor("Wt", [KCH, 128, P], dt.bfloat16, kind="ExternalInput").ap()
    Lw = nc.dram_tensor("Lw", [H, P], dt.bfloat16, kind="ExternalInput").ap()
    out_d = nc.dram_tensor("out", [C, BL], dt.float32, kind="ExternalOutput").ap()

    # chunk start columns
    ch_start = [0]
    for w in CH_COLS:
        ch_start.append(ch_start[-1] + w)

    loop_stt_names = []
    v_names = []
    stat_add_names = []
    tail_mm_names = []
    loop_mm_names = []
    big_mm_names = []

    with TileContext(nc) as tc:
        with (
            tc.tile_pool(name="const", bufs=1) as cp,
            tc.tile_pool(name="xs", bufs=5) as xp,
            tc.tile_pool(name="wk", bufs=4) as wp,
            tc.tile_pool(name="psA", bufs=8, space="PSUM") as psA,
        ):
            # ---- persistent tiles ----
            M_t = cp.tile([P, NCOL + 2 * BL], dt.bfloat16, tag="M")    # blocks 0..251
            W_t = cp.tile([128, KCH * P], dt.bfloat16, tag="W")
            L_t = cp.tile([H, P], dt.bfloat16, tag="L")
            cu0 = cp.tile([P, BL], dt.float32, tag="cu0")
            S_t = cp.tile([P, BL], dt.float32, tag="S")
            R_t = cp.tile([P, BL], dt.float32, tag="R")

            # ---- prologue: weights, inits ----
            nc.sync.dma_start(
                out=W_t[:, :],
                in_=bass.AP(Wt.tensor, 0,
                            [[P, 128], [128 * P, KCH], [1, P]]),
            )
            nc.sync.dma_start(out=L_t[:, :], in_=Lw[:, :])

            nc.vector.memset(M_t[0:H, 0:BL], 1.0)     # m_{-1} = 1 (y=0)
            nc.vector.memset(M_t[H:P, 0:BL], 0.0)
            nc.vector.memset(cu0[0:H, :], 0.0)        # v1 carry starts at 0
            nc.gpsimd.memset(S_t[H:P, :], 0.0)        # output statistic
            nc.vector.memset(cu0[H:P, :], VTH_INIT)   # kill phantom LIF2 step

            # ---- x DMAs: one k-major tile and one descriptor per chunk
            # (each dma_start costs ~730ns on the serial SP queue) ----
            xtiles = {}

            def emit_xdma(c):
                c0, w = ch_start[c], CH_COLS[c]
                t = xp.tile([128, KCH * 1024], dt.bfloat16, tag="xc")
                nc.sync.dma_start(
                    out=t[:, 0:KCH * w],
                    in_=bass.AP(xT.tensor, c0,
                                [[NCOLP, 128], [128 * NCOLP, KCH], [1, w]]),
                )
                xtiles[c] = (t, w)

            # ---- big matmul, 512-col psA blocks ----
            pa_tiles = {}

            def bigmm_block_ops(q):
                """Yield 6 matmul thunks computing psA block q (one bank,
                up to 512 cols = 16 interleaved steps)."""
                col0 = BLK_S[q] * BL
                nw = BLK_W[q] * BL
                c = next(i for i in range(len(CH_BLOCKS))
                         if q in CH_BLOCKS[i])
                n0 = col0 - sum(CH_COLS[:c])
                pa = psA.tile([P, 512], dt.float32, tag="pa")
                pa_tiles[q] = (pa, BLK_W[q])

                def mk_mm(k, pa=pa, n0=n0, nw=nw, c=c):
                    def f():
                        t, w = xtiles[c]
                        i = nc.tensor.matmul(
                            out=pa[:, 0:nw],
                            lhsT=W_t[:, k * P:(k + 1) * P],
                            rhs=t[:, k * w + n0:k * w + n0 + nw],
                            start=(k == 0), stop=False,
                        )
                        big_mm_names.append(i.ins.name)
                    return f

                for k in range(KCH):
                    yield mk_mm(k)

            # ---- interleave schedule ----
            # Spread each generation's 24 bigmm matmuls ~2.4 iterations
            # apart across the whole previous generation: a 512-col matmul
            # (~650ns) overflows one iteration's PE slack (~270ns) but the
            # debt drains before the next one arrives.
            extras = {}
            SPREAD = (0, 2, 5, 7, 10, 12)
            for g in range(1, NGENS):
                for r in range(NBGEN):
                    q = NBGEN * g + r
                    for i, th in enumerate(bigmm_block_ops(q)):
                        extras.setdefault(
                            GEN * (g - 1) + 6 + 14 * r + SPREAD[i],
                            []).append(th)
            extras.setdefault(38, []).append(lambda: emit_xdma(9))

            # prologue: x chunks 0..4, psA blocks 0..3, then chunks 5..8
            # (slot reuse needs the readers of chunks 0..3 emitted first).
            for c in range(5):
                emit_xdma(c)
            for q in range(NBGEN):
                for th in bigmm_block_ops(q):
                    th()
            for c in range(5, 9):
                emit_xdma(c)

            # ---- the sequential LIF loop ----
            cu_prev = cu0
            for j in range(TSTEPS):
                for th in extras.pop(j, []):
                    th()
                # stat add on the (otherwise idle) Pool engine: a 4th DVE
                # op per iteration throttles the DVE issue pipe to ~630ns.
                if 19 <= j:
                    bk = j - 2      # stat blocks 17..248 inside the loop
                    i4 = nc.gpsimd.tensor_tensor(
                        out=S_t[H:P, :], in0=S_t[H:P, :],
                        in1=M_t[H:P, ts(bk, BL)], op=OP.add,
                    )
                    stat_add_names.append(i4.ins.name)
                q, c = _slice_of(j)
                pa, wq = pa_tiles[q]
                ps = pa[:, c * BL:(c + 1) * BL]
                # stop on the bank's last slice only (stop flushes the whole
                # bank); CoreSim's read lint wants stop before psum reads,
                # so the sim build stops every slice (no data effect).
                im = nc.tensor.matmul(
                    out=ps, lhsT=L_t[:, :], rhs=M_t[0:H, ts(j, BL)],
                    start=False, stop=SIM_SAFE_STOPS or c == wq - 1,
                    skip_group_check=(c != 0),
                )
                loop_mm_names.append(im.ins.name)
                # qsum = 1 - (drive + recurrent); spike mask straight off
                # PSUM in ONE fused op:  v < 1  <=>  0.5*cu < qsum.
                i1_ = nc.vector.scalar_tensor_tensor(
                    out=M_t[:, ts(j + 1, BL)], in0=cu_prev[:, :], scalar=0.5,
                    in1=ps, op0=OP.mult, op1=OP.is_lt,
                )
                loop_stt_names.append(i1_.ins.name)
                if j < T:
                    v = wp.tile([P, BL], dt.float32, tag="v")
                    # u = v - 1 = 0.5*cu - qsum
                    i2 = nc.vector.scalar_tensor_tensor(
                        out=v[:, :], in0=cu_prev[:, :], scalar=0.5,
                        in1=ps, op0=OP.mult, op1=OP.subtract,
                    )
                    cu = wp.tile([P, BL], dt.float32, tag="cu")
                    # cu = v*m = (u + 1)*m
                    i3 = nc.vector.scalar_tensor_tensor(
                        out=cu[:, :], in0=v[:, :], scalar=1.0,
                        in1=M_t[:, ts(j + 1, BL)], op0=OP.add, op1=OP.mult,
                    )
                    loop_stt_names.extend([i2.ins.name, i3.ins.name])
                    v_names.append(i2.ins.name)
                    cu_prev = cu
            for jj in sorted(extras):
                for th in extras[jj]:
                    th()

            # ---- tail: last stat blocks, then the output scale ----
            for bk in (249, 250, 251):
                nc.gpsimd.tensor_tensor(
                    out=S_t[H:P, :], in0=S_t[H:P, :],
                    in1=M_t[H:P, ts(bk, BL)], op=OP.add,
                )
            nc.gpsimd.tensor_scalar(
                out=R_t[H:P, :], in0=S_t[H:P, :],
                scalar1=235.0, scalar2=-1.0 / 235.0,
                op0=OP.subtract, op1=OP.mult,
            )
            nc.sync.dma_start(out=out_d[:, :], in_=R_t[H:P, :])

    nc._loop_stt_names = loop_stt_names + stat_add_names
    nc._v_names = v_names
    nc._tail_mm_names = tail_mm_names
    nc._loop_mm_names = loop_mm_names
    nc._big_mm_names = big_mm_names
    _fix_sync(nc)
    return nc


def _eng_of(w):
    n = w.ant_name
    if "DVE" in n:
        return "DVE"
    if "Activation" in n:
        return "ACT"
    if "PE" in n:
        return "PE"
    if "Pool" in n:
        return "POOL"
    return "OTHER"


def _fix_sync(nc):
    """walrus accepts only ONE sync wait per compute instruction (AC/MM/STT).
    Keep, per instruction, the single wait that engine-order transitivity
    cannot cover:

      * loop STTs (mask/v/cu) and stat adds: the mask keeps its PE (psum)
        wait; the others drop their same-engine self-waits (DVE executes in
        order, and consecutive [84,32] ops observe each other's writes at
        issue cadence -- the baseline already relied on this for mask<-cu).
      * loop matmuls: keep the DVE wait (recurrent-mask RAW).  The psum
        accumulation group ordering vs the bigmm matmuls is PE-in-order.
      * bigmm matmuls: keep the DMA (xtile) wait.  The psA-slot WAR vs the
        DVE readers of 4 blocks ago is covered by the kept DVE waits of the
        loop matmuls that precede this matmul in the PE stream (DVE
        semaphore counts are completion-ordered), with ~50 periods margin.
      * tail bias matmul: keep the DVE (ones2 memset) wait; its weight DMA
        is covered by the prologue LDWEIGHTS of the same tile.
      * Drains keep only output-DMA lanes (input-DMA completions are covered
        by their consumers' waits; engine completion by the final barrier).
    """
    import concourse.mybir as mybir

    tail_mm = set(nc._tail_mm_names)
    loop_mm = set(nc._loop_mm_names)
    big_mm = set(nc._big_mm_names)
    loop_stt = set(nc._loop_stt_names)

    out_names = set()
    for alloc in nc.m.functions[0].allocations:
        if (
            isinstance(alloc, mybir.MemoryLocationSet)
            and alloc.kind == "ExternalOutput"
        ):
            for ml in alloc.memorylocations:
                out_names.add(ml.name)
    keep_lanes = set()
    for name, inst in nc.inst_map.items():
        if "DMA" not in type(inst).__name__:
            continue
        c = inst.concise()
        if any(f"@{n}" in c.split("in=")[0] for n in out_names):
            for u in (inst.sync_info.on_update or []) if inst.sync_info else []:
                keep_lanes.add(u.ant_name)

    problems = []
    for name, inst in nc.inst_map.items():
        si = inst.sync_info
        if si is None or not si.on_wait:
            continue
        waits = list(si.on_wait)
        own = {u.ant_name for u in (si.on_update or [])}

        if name in loop_mm or name in tail_mm:
            kept = [w for w in waits if _eng_of(w) == "DVE"]
            if not kept:
                kept = [w for w in waits if w.ant_name not in own]
        elif name in big_mm:
            kept = [w for w in waits
                    if _eng_of(w) not in ("DVE", "ACT")
                    and w.ant_name not in own]
            if not kept and waits:
                kept = [w for w in waits if w.ant_name not in own][:1]
        elif name in loop_stt:
            kept = [w for w in waits if w.ant_name not in own]
        elif len(waits) >= 2:
            kept = [w for w in waits if w.ant_name not in own]
            if "Drain" in type(inst).__name__ and len(kept) > 1:
                kept = [w for w in kept if w.ant_name in keep_lanes]
            if "DMACopy" in type(inst).__name__ and len(kept) > 1:
                # DMA_DIRECT2D takes a single wait.  The PE wait (WAR vs the
                # slot's previous readers) subsumes the DMA-lane WAW: those
                # readers only executed after the previous transfer landed.
                pe = [w for w in kept if _eng_of(w) == "PE"]
                kept = pe if pe else kept[:1]
        else:
            continue

        if len(kept) != len(waits):
            si.on_wait = kept
        if len(kept) > 1 and "Drain" not in type(inst).__name__ \
                and "DMA" not in type(inst).__name__ \
                and "Branch" not in type(inst).__name__:
            problems.append((name, type(inst).__name__,
                             [w.ant_name for w in kept]))
    if problems:
        for p in problems[:8]:
            print("MULTIWAIT:", p)

def _prep_shared(W1, b1, Wr, br, W2, b2):
    f32 = np.float32
    W1 = np.asarray(W1, f32); b1 = np.asarray(b1, f32)
    Wr = np.asarray(Wr, f32); br = np.asarray(br, f32)
    W2 = np.asarray(W2, f32); b2 = np.asarray(b2, f32)
    Wrh, Wry = Wr[:, :H], Wr[:, H:]
    # Negated ("qsum = 1 - v") encoding: psum = (1-bt) - Wtil@x
    # - 0.5*[Wry;W2]@m with m in {0,1}; spike test is then 0.5*cu < q.
    Wtil = -0.5 * (Wrh @ W1)                                  # [64, 700]
    bt1 = 0.5 * (Wrh @ b1 + br + Wry.sum(axis=1))
    bt2 = 0.5 * (b2 + W2.sum(axis=1))
    bfl = 1.0 - np.concatenate([bt1, bt2])                    # [84] fp32
    Wtp = np.zeros((P, DP), f32)
    Wtp[:H, :D] = Wtil
    # Bias rides two spare contraction rows as a double-bf16 split; the
    # matching x rows are constant 1.0, so psum picks up ~fp32 bias.
    bhi = bfl.astype(BF16).astype(f32)
    Wtp[:, BIAS_ROW] = bhi
    Wtp[:, BIAS_ROW + 1] = bfl - bhi
    Wt6 = np.ascontiguousarray(
        Wtp.reshape(P, KCH, 128).transpose(1, 2, 0)
    ).astype(BF16)                                            # [6, 128, 84]
    L = np.concatenate([0.5 * Wry.T, 0.5 * W2.T], axis=1).astype(BF16)
    return Wt6, L


def _core_xt(xc_bf):
    """Build one core's permuted drive matrix [KCH, 128, NCOLP] from its
    (BL, T, D) bf16 input slice: step j lands in column group BLK_S[q]+c
    (the bank-interleaved layout), bias rows are constant 1."""
    xt = np.zeros((DP, NCOLP // BL, BL), BF16)
    gidx = np.empty(TSTEPS, np.int64)
    for j in range(TSTEPS):
        q, c = _slice_of(j)
        gidx[j] = BLK_S[q] + c
    xt[:D, gidx[:T]] = xc_bf.transpose(2, 1, 0)
    xt[BIAS_ROW:BIAS_ROW + 2] = 1.0
    return np.ascontiguousarray(xt.reshape(KCH, 128, NCOLP))


def _ensure_ntff_hook():
    """The RL container's antenv stub lacks axon_hooks; bass_utils imports it
    unconditionally when tracing. Register the ctypes-based hook ourselves."""
    import sys
    import types
    try:
        import antenv
        if "antenv.axon_hooks" in sys.modules:
            return
        mod = types.ModuleType("antenv.axon_hooks")
        _h = [None]
        mod.set_axon_ntff_profile_hook = lambda h: _h.__setitem__(0, h)
        mod.get_axon_ntff_profile_hook = lambda: _h[0]
        sys.modules["antenv.axon_hooks"] = mod
        antenv.axon_hooks = mod
        try:
            from trn_agent_boot.trn_boot import _ntff_profile_via_ctypes
            mod.set_axon_ntff_profile_hook(
                _ntff_profile_via_ctypes("/opt/axon/libaxon_pjrt.so")
            )
        except Exception:
            pass
    except Exception:
        pass


def kernel(x, W1, b1, Wr, br, W2, b2):
    from concourse.bass_utils import run_bass_kernel_spmd

    _ensure_ntff_hook()

    if "nc" not in _CACHE:
        _CACHE["nc"] = _build_nc()
    nc = _CACHE["nc"]

    Wt6, L = _prep_shared(W1, b1, Wr, br, W2, b2)

    x = np.asarray(x, np.float32)
    xbf = x.astype(BF16)                                      # (B, T, D)
    in_maps = []
    for c in range(NCORES):
        xc = xbf[c * BL:(c + 1) * BL]                         # (32, 250, 700)
        in_maps.append({
            "xT": _core_xt(xc), "Wt": Wt6, "Lw": L,
        })

    res = run_bass_kernel_spmd(nc, in_maps, core_ids=list(range(NCORES)))
    _CACHE["last_results"] = res
    out = np.concatenate(
        [np.asarray(r["out"]).T for r in res.results], axis=0
    ).astype(np.float32)                                      # (256, 20)
    return out
